# revision 18
# baseline (speedup 1.0000x reference)
"""Trainium2 Bass kernel for a 2-state linear-chain CRF loss (BiLSTM-CRF loss_fn).

Computes, for a single conversation of length T = 2,097,152:
  gold_score  = sum_t em[t, lab[t]] + sum_{t>0} trans[t][lab[t-1], lab[t]]
  total_score = logsumexp of the CRF forward recursion
where trans[t] = who2who_sub[w[t]] + position_sub[p[t]] (60 possible 2x2
matrices; indices 2/19 select an all-zero padding matrix).

Design (one NeuronCore per contiguous chunk of 262,144 steps, 8 cores):

* Per-step matrices: trans+em is built as 4 fp16 streams by per-class masked
  accumulation (19 position classes + 2 who2who classes + emission fold).
  Class supports are disjoint, so sums of masked values are exact in fp16;
  the masked values are combined PAIRWISE (a small in-group tree) so the
  per-stream dependency depth is ~8 instead of 21 serial adds.  Work is
  split three ways: DVE runs fused (idx==c)*V tensor_scalars (4x fp16 mode)
  plus most adds; the ACT engine produces masked values for a suffix of
  position classes as Relu((V+B) - (V+B)*(p-c)^2) with B=4 making the peak
  positive (the spurious +B*[p>=a] is removed by one (p>a-.5)*B mask and 4
  subtracts; the t=0 pad step ends shifted by exactly -B, corrected on the
  host); GPSIMD takes a striped share of the adds.

* Gold score: the label-pair stream msel = 2*lab[t-1]+lab[t] selects one of
  the 4 finished streams per step; gold = sum_t acc[msel_t][t] via 4
  is_equal masks + multiply + accum_out per sub-chunk.  Exactness: stream
  values are single-fp16-rounded table values (+ exact-in-fp16 shifts), and
  the host chooses each table entry's fp16 rounding DIRECTION (greedy sign
  optimization over the 19x2x4 reachable cells) so the systematic selection
  bias cancels to ~1e-5 relative.

* Forward pass: the recursion is a product of 2x2 matrices in the (log, +)
  semiring; each core tree-reduces with LSE(a,b) = a + ln(1+exp(b-a)) on
  ACT.  The chunk is split into 2 sub-chunks of 1024 steps per partition,
  emitted with a 6-block skew so sub-chunk 0's tree overlaps sub-chunk 1's
  stream build.  Each core ships its 2*128 sub-chunk matrices +
  per-partition gold; the host does the O(cores*P) ordered log-semiring
  combine (vectorized numpy).

* All inputs ship as one fp16 blob [par | p | w | msel | em0 | em1] in 4
  DMAs (params + the first sub-chunk's position stream lead) so the class
  masks start immediately.
"""

from contextlib import ExitStack

import numpy as np

import concourse.bass as bass
import concourse.bacc as bacc
import concourse.mybir as mybir
import concourse.tile as tile
from concourse import bass_utils

dt = mybir.dt
ALU = mybir.AluOpType
AF = mybir.ActivationFunctionType
AX = mybir.AxisListType

T = 2097152
NCORES = 8
P = 128                  # SBUF partitions
L = T // NCORES          # steps per core = 262144
F = L // P               # steps per partition = 2048
SC_SIZES = (1024, 512, 512)
SC = len(SC_SIZES)
NPOS = 19                # position classes with nonzero matrices
BSH = 8.0                # ACT positivity shift
ACT_LO = 9               # position classes >= this use ACT-produced mv
EW = 4 * SC + 1          # out row: SC matrices (4 entries each) + gold

# param row layout (f32 words): [pos' 19*4 | D 4 | VB 19*4 | negVB 19*4]
# pos' = pos + w2w[1] (folded);  D = w2w[0] - w2w[1]
NPAR = 19 * 4 + 4 + 19 * 4 + 19 * 4
COL_POS = 0
COL_W = 76
COL_VB = 80
COL_NVB = 156

W0 = 2 * NPAR + 5 * F    # fp16 blob columns


_NC_CACHE = None
LAST_RESULTS = None  # BassKernelResults of the most recent kernel() call


def _comp(i, j):
    return i * 2 + j


def _build_nc():
    nc = bacc.Bacc()

    b0_d = nc.dram_tensor("blob0", [P, W0], dt.float16, kind="ExternalInput")
    out_d = nc.dram_tensor("out", [P, EW], dt.float32, kind="ExternalOutput")

    with ExitStack() as ctx:
        tc = ctx.enter_context(tile.TileContext(nc))
        pool = ctx.enter_context(tc.tile_pool(name="main", bufs=1))

        # ---- loads: [par | p | w | msel | em0 | em1] in 3 DMAs ----
        b0 = pool.tile([P, W0], dt.float16, tag="b0", name="b0")
        hq = 2 * NPAR + 1024       # par + p columns for sub-chunk 0
        h0 = 2 * NPAR + F          # par + p
        h1 = h0 + 2 * F            # + w + msel
        nc.sync.dma_start(b0[:, 0:hq], b0_d[:, 0:hq])
        nc.sync.dma_start(b0[:, hq:h0], b0_d[:, hq:h0])
        nc.sync.dma_start(b0[:, h0:h1], b0_d[:, h0:h1])
        nc.sync.dma_start(b0[:, h1:W0], b0_d[:, h1:W0])

        par32 = b0[:, 0:2 * NPAR].bitcast(dt.float32)
        p_t = b0[:, 2 * NPAR:h0]
        w_t = b0[:, h0:h0 + F]
        msel_t = b0[:, h0 + F:h1]
        em0_t = b0[:, h1:h1 + F]
        em1_t = b0[:, h1 + F:W0]

        def V(col):
            return par32[:, col:col + 1]

        bias_c = {}
        for c in range(ACT_LO, NPOS):
            t_ = pool.tile([P, 1], dt.float32, tag=f"bc{c}", name=f"bc{c}")
            nc.vector.memset(t_[:], -float(c))
            bias_c[c] = t_
        bconst = pool.tile([P, 1], dt.float32, tag="bconst", name="bconst")
        nc.vector.memset(bconst[:], BSH)

        FSMAX = max(SC_SIZES)
        HSMAX = FSMAX // 2
        SC_OFF = [sum(SC_SIZES[:i]) for i in range(SC)]
        acc = [
            pool.tile([P, F], dt.float16, tag=f"acc{c}", name=f"acc{c}")
            for c in range(4)
        ]
        # mv work tiles, full-F, sliced per sub-chunk (cross-SC skew spaces
        # same-SC reuse)
        mv = [
            pool.tile([P, F], dt.float16, tag=f"mv{i}", name=f"mv{i}")
            for i in range(4 * 4)
        ]

        def mv_t(ki, c, s):
            return mv[ki * 4 + c][:, SC_OFF[s]:SC_OFF[s] + SC_SIZES[s]]

        amv = [
            pool.tile([P, F], dt.float16, tag=f"amv{i}", name=f"amv{i}")
            for i in range(3 * 4)
        ]

        def amv_t(ki, c, s):
            return amv[ki * 4 + c][:, SC_OFF[s]:SC_OFF[s] + SC_SIZES[s]]

        sqt = [
            pool.tile([P, F], dt.float16, tag=f"sq{i}", name=f"sq{i}")
            for i in range(2)
        ]
        mvb = pool.tile([P, FSMAX], dt.float16, tag="mvb", name="mvb")
        qm = [
            pool.tile([P, FSMAX], dt.float16, tag=f"qm{i}", name=f"qm{i}")
            for i in range(2)
        ]
        gprod = [
            pool.tile([P, FSMAX], dt.float16, tag=f"gp{i}", name=f"gp{i}")
            for i in range(2)
        ]
        gcell = pool.tile([P, 4 * SC], dt.float32, tag="gcell", name="gcell")
        res = pool.tile([P, EW], dt.float32, tag="res", name="res")

        # striped DVE/Pool assignment for accumulate adds
        POOL_NUM, POOL_DEN = 1, 3
        add_ctr = [0] * 4

        def add_eng(comp):
            add_ctr[comp] += 1
            k = (add_ctr[comp] + comp) % POOL_DEN
            return nc.gpsimd if k < POOL_NUM else nc.vector

        def tadd(comp, out, a, b):
            add_eng(comp).tensor_add(out, a, b)

        def sc_views(s):
            sl = slice(SC_OFF[s], SC_OFF[s] + SC_SIZES[s])
            return (p_t[:, sl], w_t[:, sl], msel_t[:, sl],
                    [a[:, sl] for a in acc], sl)

        dve_classes = list(range(0, ACT_LO))
        dgroups = [dve_classes[i:i + 4]
                   for i in range(0, len(dve_classes), 4)]
        act_classes = list(range(ACT_LO, NPOS))
        agroups = [act_classes[i:i + 3]
                   for i in range(0, len(act_classes), 3)]

        def emit_dve_group(s, gi):
            grp = dgroups[gi]
            p_s, w_s, m_s, acc_s, sl = sc_views(s)
            FS = SC_SIZES[s]
            for c in range(4):
                tiles = []
                for ki, k in enumerate(grp):
                    m = mv_t(ki, c, s)
                    nc.vector.tensor_scalar(
                        m, p_s, float(k), V(COL_POS + 4 * k + c),
                        ALU.is_equal, ALU.mult,
                    )
                    tiles.append(m)
                if len(tiles) == 4:
                    tadd(c, tiles[0], tiles[0], tiles[1])
                    tadd(c, tiles[2], tiles[2], tiles[3])
                    if gi == 0:
                        tadd(c, acc_s[c], tiles[0], tiles[2])
                    else:
                        tadd(c, tiles[0], tiles[0], tiles[2])
                        tadd(c, acc_s[c], acc_s[c], tiles[0])
                else:
                    while len(tiles) > 1:
                        tadd(c, tiles[0], tiles[0], tiles[1])
                        tiles = [tiles[0]] + tiles[2:]
                    if gi == 0:
                        nc.vector.tensor_copy(acc_s[c], tiles[0])
                    else:
                        tadd(c, acc_s[c], acc_s[c], tiles[0])

        def emit_act_group(s, gi):
            grp = agroups[gi]
            p_s, w_s, m_s, acc_s, sl = sc_views(s)
            FS = SC_SIZES[s]
            for ki, k in enumerate(grp):
                sq = sqt[ki % 2][:, SC_OFF[s]:SC_OFF[s] + FS]
                nc.scalar.activation(sq, p_s, AF.Square, bias=bias_c[k][:])
                for c in range(4):
                    nc.scalar.activation(
                        amv_t(ki, c, s), sq, AF.Relu,
                        bias=V(COL_VB + 4 * k + c),
                        scale=V(COL_NVB + 4 * k + c),
                    )
            for c in range(4):
                tiles = [amv_t(ki, c, s) for ki in range(len(grp))]
                while len(tiles) > 1:
                    tadd(c, tiles[0], tiles[0], tiles[1])
                    tiles = [tiles[0]] + tiles[2:]
                tadd(c, acc_s[c], acc_s[c], tiles[0])

        def emit_sub_w_em(s):
            p_s, w_s, m_s, acc_s, sl = sc_views(s)
            FS = SC_SIZES[s]
            # remove the spurious +B over [p >= ACT_LO] (includes the t=0
            # pad step p=19; host adds B back to both outputs)
            nc.vector.tensor_scalar(
                mvb[:, 0:FS], p_s, ACT_LO - 0.5, bconst[:], ALU.is_gt,
                ALU.mult,
            )
            for c in range(4):
                add_eng(c).tensor_sub(acc_s[c], acc_s[c], mvb[:, 0:FS])
            # who2who: w2w[1] is folded into the position table, so only
            # (w==0)*(w2w[0]-w2w[1]) remains (w==2 occurs only at t=0)
            for c in range(4):
                m0 = mv_t(0, c, s)
                nc.vector.tensor_scalar(
                    m0, w_s, 0.0, V(COL_W + c), ALU.is_equal, ALU.mult,
                )
                tadd(c, acc_s[c], acc_s[c], m0)
            # emission fold: acc[i,j] += em_j
            for c in range(4):
                em_s = (em0_t if c % 2 == 0 else em1_t)[:, sl]
                tadd(c, acc_s[c], acc_s[c], em_s)

        def emit_gold(s):
            p_s, w_s, m_s, acc_s, sl = sc_views(s)
            FS = SC_SIZES[s]
            for pair in range(4):
                q = qm[pair % 2][:, 0:FS]
                nc.vector.tensor_scalar(q, m_s, float(pair), None,
                                        ALU.is_equal)
                pr = gprod[pair % 2][:, 0:FS]
                nc.vector.tensor_mul(pr, q, acc_s[pair])
                nc.vector.tensor_scalar(
                    pr, pr, 1.0, None, ALU.mult, ALU.add,
                    accum_out=gcell[:, s * 4 + pair:s * 4 + pair + 1],
                )

        def u2(ap):
            return ap.unsqueeze(2).unsqueeze(3)

        def emit_tree(s):
            p_s, w_s, m_s, acc_s, sl = sc_views(s)
            FS = SC_SIZES[s]
            HS = FS // 2
            FP16_LEVELS = 4
            X16 = pool.tile([P, HSMAX, 2, 2], dt.float16, tag="X16",
                            name="X16")[:, 0:HS]
            Y16a = pool.tile([P, HSMAX, 2, 2], dt.float16, tag="Y16a",
                             name="Y16a")[:, 0:HS]
            Y16b = pool.tile([P, HSMAX // 2, 2, 2], dt.float16, tag="Y16b",
                             name="Y16b")[:, 0:HS // 2]
            X32 = pool.tile([P, HSMAX // 16, 2, 2], dt.float32, tag="X32",
                            name="X32")[:, 0:max(HS // 16, 1)]
            Y0 = pool.tile([P, HSMAX, 2, 2], dt.float32, tag="Y0",
                           name="Y0")[:, 0:HS]
            Y1 = pool.tile([P, HSMAX // 2, 2, 2], dt.float32, tag="Y1",
                           name="Y1")[:, 0:HS // 2]
            for i in range(2):
                for j in range(2):
                    add_eng(_comp(i, j)).tensor_add(
                        X16[:, :, i:i + 1, j:j + 1],
                        u2(acc_s[_comp(i, 0)][:, 0::2]),
                        u2(acc_s[_comp(0, j)][:, 1::2]),
                    )
                    add_eng(_comp(i, j)).tensor_add(
                        Y16a[:, :, i:i + 1, j:j + 1],
                        u2(acc_s[_comp(i, 1)][:, 0::2]),
                        u2(acc_s[_comp(1, j)][:, 1::2]),
                    )
            nc.vector.tensor_sub(Y16a[:], Y16a[:], X16[:])
            nc.scalar.activation(Y0[:], Y16a[:], AF.Exp)
            nc.scalar.activation(Y0[:], Y0[:], AF.Ln, bias=1.0)
            mlev = pool.tile([P, HSMAX, 2, 2], dt.float16, tag="m1",
                             name="m1")[:, 0:HS]
            nc.vector.tensor_add(mlev[:], X16[:], Y0[:])

            w_cur = HS
            lev = 1
            while w_cur > 1:
                w2 = w_cur // 2
                lev += 1
                sh = [P, w2, 2, 2]
                a_i0 = mlev[:, 0:w_cur:2, :, 0:1].broadcast_to(sh)
                a_i1 = mlev[:, 0:w_cur:2, :, 1:2].broadcast_to(sh)
                b_0j = mlev[:, 1:w_cur:2, 0:1, :].broadcast_to(sh)
                b_1j = mlev[:, 1:w_cur:2, 1:2, :].broadcast_to(sh)
                sp = (Y0 if lev % 2 == 1 else Y1)[:, 0:w2]
                if lev <= FP16_LEVELS:
                    xv = X16[:, 0:w2]
                    yv = (Y16a if lev % 2 == 1 else Y16b)[:, 0:w2]
                    (add_eng(lev % 4) if w2 >= 64 else nc.vector).tensor_add(
                        xv, a_i0, b_0j)
                    (add_eng((lev + 2) % 4) if w2 >= 64 else nc.vector
                     ).tensor_add(yv, a_i1, b_1j)
                    nc.vector.tensor_sub(yv, yv, xv)
                    nc.scalar.activation(sp, yv, AF.Exp)
                else:
                    xv = X32[:, 0:w2]
                    yv = sp
                    nc.vector.tensor_add(xv, a_i0, b_0j)
                    nc.vector.tensor_add(yv, a_i1, b_1j)
                    nc.vector.tensor_sub(yv, yv, xv)
                    nc.scalar.activation(sp, sp, AF.Exp)
                nc.scalar.activation(sp, sp, AF.Ln, bias=1.0)
                mdt = dt.float16 if lev <= FP16_LEVELS else dt.float32
                mwidth = max(HSMAX // (2 ** (lev - 1)), 1)
                mnext = pool.tile([P, mwidth, 2, 2], mdt, tag=f"m{lev}",
                                  name=f"m{lev}")[:, 0:w2]
                nc.vector.tensor_add(mnext[:], xv, sp)
                mlev = mnext
                w_cur = w2
            nc.vector.tensor_copy(
                res[:, 4 * s:4 * s + 4].rearrange("p (a b c) -> p a b c",
                                                  a=1, b=2),
                mlev[:],
            )

        # ---- skewed block emission: sub-chunk s trails s-1 by SKEW blocks
        # so completions stagger and each tree overlaps the next build ----
        SKEW = 4
        blocks = []
        for s in range(SC):
            seq = []
            seq.append(lambda s=s: emit_dve_group(s, 0))
            for gi in range(len(agroups)):
                seq.append(lambda s=s, gi=gi: emit_act_group(s, gi))
                if gi + 1 < len(dgroups):
                    seq.append(lambda s=s, gi=gi: emit_dve_group(s, gi + 1))
            seq.append(lambda s=s: emit_sub_w_em(s))
            seq.append(lambda s=s: emit_gold(s))
            seq.append(lambda s=s: emit_tree(s))
            for bi, fn in enumerate(seq):
                blocks.append((bi + SKEW * s, s, fn))
        blocks.sort(key=lambda kv: (kv[0], kv[1]))
        for _, _, fn in blocks:
            fn()

        # ---- gold column and store; host combines ----
        nc.vector.reduce_sum(res[:, 4 * SC:4 * SC + 1], gcell[:], axis=AX.X)
        nc.sync.dma_start(out_d[:], res[:])

    nc.compile()

    # Exp/Ln/Square/Relu all live in 'natural_log_exp_and_others', but
    # insert_act_table_loads picks the first set containing each function,
    # emitting alternating table reloads (1.3 us each).  Retarget every load
    # to the combined set and drop the now-redundant ones.
    from concourse.hw_specs import get_activation_tables

    tables = list(get_activation_tables(nc.m.arch).keys())
    combined = tables.index("natural_log_exp_and_others")
    for b in nc.bb_map.values():
        insts = b.bb.instructions
        kept = []
        seen_load = False
        for ins in insts:
            if ins.opcode == "LoadActFuncSet":
                si = ins.sync_info
                assert not (si and (si.on_wait or si.on_update)), ins.name
                if seen_load:
                    continue
                ins.act_func_set_id = combined
                seen_load = True
            kept.append(ins)
        if len(kept) != len(insts):
            b.bb.instructions = kept
    return nc


def _get_nc():
    global _NC_CACHE
    if _NC_CACHE is None:
        _NC_CACHE = _build_nc()
    return _NC_CACHE


def _f16_candidates(x, grid_pow=None):
    """Nearest fp16 (or 2^grid_pow-grid) value and its other-side neighbor."""
    if grid_pow is None:
        lo = np.float16(x)
        res = float(x) - float(lo)
        if res == 0.0:
            return np.float32(lo), np.float32(lo)
        hi = np.nextafter(lo, np.float16(np.inf if res > 0 else -np.inf),
                          dtype=np.float16)
        return np.float32(lo), np.float32(hi)
    g = 2.0 ** grid_pow
    lo = np.floor(float(x) / g) * g
    hi = lo + g
    if abs(float(x) - lo) <= abs(hi - float(x)):
        return np.float32(lo), np.float32(hi)
    return np.float32(hi), np.float32(lo)


def _optimize_tables(pos, w2w):
    """fp16 tables with per-entry rounding direction chosen so the
    systematic selected-sum bias over the reachable (p,w) cells cancels.
    w2w[1] is folded into the position table (P' = pos + w2w[1]); the
    residual class value is D = w2w[0] - w2w[1].  ACT-suffix rows sit on
    the 2^-7 grid so V+BSH stays fp16-exact."""
    posr = pos.reshape(NPOS, 4).astype(np.float64)
    wr = w2w.reshape(2, 4).astype(np.float64)
    Pp = posr + wr[1]
    Dv = wr[0] - wr[1]

    Pc = np.zeros((NPOS, 4, 2), np.float32)
    for k in range(NPOS):
        gp = -7 if k >= ACT_LO else None
        for c in range(4):
            Pc[k, c] = _f16_candidates(Pp[k, c], gp)
    Dc = np.zeros((4, 2), np.float32)
    for c in range(4):
        Dc[c] = _f16_candidates(Dv[c])

    # cell (k, w=1): value = P16[k];  cell (k, w=0): fp16(P16[k] + D16)
    e1 = posr + wr[1]
    e0 = posr + wr[0]
    d1 = Pc.astype(np.float64) - e1[:, :, None]               # [19,4,2]
    d0 = ((Pc[:, :, :, None].astype(np.float16)
           + Dc[None, :, None, :].astype(np.float16)).astype(np.float16)
          .astype(np.float64) - e0[:, :, None, None])          # [19,4,2,2]

    Ps = np.zeros((NPOS, 4), np.intp)
    Ds = np.zeros(4, np.intp)

    def total():
        s = 0.0
        for k in range(NPOS):
            for c in range(4):
                s += d1[k, c, Ps[k, c]] + d0[k, c, Ps[k, c], Ds[c]]
        return s

    best = total()
    for _ in range(4):
        improved = False
        for k in range(NPOS):
            for c in range(4):
                Ps[k, c] ^= 1
                t2 = total()
                if abs(t2) < abs(best):
                    best = t2
                    improved = True
                else:
                    Ps[k, c] ^= 1
        for c in range(4):
            Ds[c] ^= 1
            t2 = total()
            if abs(t2) < abs(best):
                best = t2
                improved = True
            else:
                Ds[c] ^= 1
        if not improved:
            break

    P16 = np.take_along_axis(Pc, Ps[:, :, None], axis=2)[:, :, 0]
    D16 = Dc[np.arange(4), Ds]
    return P16.astype(np.float32), D16.astype(np.float32)


def _lse_combine(A, B):
    """ordered log-semiring 2x2 product, vectorized over leading dims"""
    return np.logaddexp(A[..., :, 0:1] + B[..., 0:1, :],
                        A[..., :, 1:2] + B[..., 1:2, :])


def kernel(**inputs):
    em = np.asarray(inputs["emission_scores"], dtype=np.float32)
    lab = np.asarray(inputs["label"]).astype(np.float32)
    w = np.asarray(inputs["who2who_state"]).astype(np.float32)
    p = np.asarray(inputs["position_state"]).astype(np.float32)
    w2w = np.asarray(inputs["who2who_params"], dtype=np.float32)
    pos = np.asarray(inputs["position_params"], dtype=np.float32)
    assert em.shape == (T, 2), em.shape

    labp = np.empty_like(lab)
    labp[0] = 0.0
    labp[1:] = lab[:-1]
    msel = (2.0 * labp + lab).astype(np.float16)

    P16, D16 = _optimize_tables(pos, w2w)
    vb = P16 + np.float32(BSH)   # fp16-exact for the ACT rows (2^-7 grid)
    par_row = np.concatenate([
        P16.reshape(-1), D16.reshape(-1), vb.reshape(-1), (-vb).reshape(-1)
    ]).astype(np.float32)
    assert par_row.shape[0] == NPAR
    par16 = np.broadcast_to(par_row.view(np.float16), (P, 2 * NPAR))

    p16 = p.astype(np.float16)
    w16 = w.astype(np.float16)
    em16 = em.astype(np.float16)

    in_maps = []
    for k in range(NCORES):
        sl = slice(k * L, (k + 1) * L)
        blob0 = np.concatenate(
            [
                par16,
                p16[sl].reshape(P, F),
                w16[sl].reshape(P, F),
                msel[sl].reshape(P, F),
                np.ascontiguousarray(em16[sl, 0].reshape(P, F)),
                np.ascontiguousarray(em16[sl, 1].reshape(P, F)),
            ],
            axis=1,
        )
        in_maps.append({"blob0": np.ascontiguousarray(blob0)})

    nc = _get_nc()
    kr = bass_utils.run_bass_kernel_spmd(nc, in_maps, core_ids=list(range(NCORES)))
    global LAST_RESULTS
    LAST_RESULTS = kr
    results = kr.results

    # host combine: ordered product of NCORES*P*SC 2x2 matrices + gold sum
    rows = np.stack([np.asarray(r["out"], dtype=np.float64) for r in results])
    gold = rows[:, :, 4 * SC].sum()
    mats = rows[:, :, 0:4 * SC].reshape(NCORES * P * SC, 2, 2)
    # pairwise tree keeps it fast and stable
    while mats.shape[0] > 1:
        n = mats.shape[0]
        even = mats[0:n - 1:2]
        odd = mats[1:n:2]
        comb = _lse_combine(even, odd)
        if n % 2 == 1:
            comb = np.concatenate([comb, mats[n - 1:n]], axis=0)
        mats = comb
    total = np.logaddexp.reduce(mats.reshape(-1))
    # the single t=0 pad step (p=19) carries the -BSH shift: add it back
    gold += BSH
    total += BSH
    return np.stack([gold, total]).astype(np.float32)


if __name__ == "__main__":
    rng = np.random.default_rng(0)
    demo = dict(
        emission_scores=rng.standard_normal((T, 2)).astype(np.float32),
        label=rng.integers(0, 2, T),
        who2who_state=np.concatenate([[2], rng.integers(0, 2, T - 1)]),
        position_state=np.concatenate([[19], rng.integers(0, 19, T - 1)]),
        who2who_params=rng.standard_normal((2, 2, 2)).astype(np.float32),
        position_params=rng.standard_normal((19, 2, 2)).astype(np.float32),
    )
    print(kernel(**demo))


# revision 29
# speedup vs baseline: 1.0162x; 1.0162x over previous
"""Trainium2 Bass kernel for a 2-state linear-chain CRF loss (BiLSTM-CRF loss_fn).

Computes, for a single conversation of length T = 2,097,152:
  gold_score  = sum_t em[t, lab[t]] + sum_{t>0} trans[t][lab[t-1], lab[t]]
  total_score = logsumexp of the CRF forward recursion
where trans[t] = who2who_sub[w[t]] + position_sub[p[t]] (60 possible 2x2
matrices; indices 2/19 select an all-zero padding matrix).

Design (one NeuronCore per contiguous chunk of 262,144 steps, 8 cores):

* Per-step matrices: trans+em is built as 4 fp16 streams by per-class masked
  accumulation (19 position classes + 2 who2who classes + emission fold).
  Class supports are disjoint, so sums of masked values are exact in fp16;
  the masked values are combined PAIRWISE (a small in-group tree) so the
  per-stream dependency depth is ~8 instead of 21 serial adds.  Work is
  split three ways: DVE runs fused (idx==c)*V tensor_scalars (4x fp16 mode)
  plus most adds; the ACT engine produces masked values for a suffix of
  position classes as Relu((V+B) - (V+B)*(p-c)^2) with B=4 making the peak
  positive (the spurious +B*[p>=a] is removed by one (p>a-.5)*B mask and 4
  subtracts; the t=0 pad step ends shifted by exactly -B, corrected on the
  host); GPSIMD takes a striped share of the adds.

* Gold score: the label-pair stream msel = 2*lab[t-1]+lab[t] selects one of
  the 4 finished streams per step; gold = sum_t acc[msel_t][t] via 4
  is_equal masks + multiply + accum_out per sub-chunk.  Exactness: stream
  values are single-fp16-rounded table values (+ exact-in-fp16 shifts), and
  the host chooses each table entry's fp16 rounding DIRECTION (greedy sign
  optimization over the 19x2x4 reachable cells) so the systematic selection
  bias cancels to ~1e-5 relative.

* Forward pass: the recursion is a product of 2x2 matrices in the (log, +)
  semiring; each core tree-reduces with LSE(a,b) = a + ln(1+exp(b-a)) on
  ACT.  The chunk is split into 3 sub-chunks of 1024/512/512 steps per
  partition so each sub-chunk's tree overlaps the next one's stream build
  and only the last (small) tree is exposed at the end.  Each core ships
  its 3*128 sub-chunk matrices + per-partition gold; the host does the
  O(cores*P) ordered log-semiring combine (vectorized numpy).

* All inputs ship as one fp16 blob [par | p | w | msel | em0 | em1] in 3
  DMAs so the class masks start immediately.
"""

from contextlib import ExitStack

import numpy as np

import concourse.bass as bass
import concourse.bacc as bacc
import concourse.mybir as mybir
import concourse.tile as tile
from concourse import bass_utils

dt = mybir.dt
ALU = mybir.AluOpType
AF = mybir.ActivationFunctionType
AX = mybir.AxisListType

T = 2097152
NCORES = 8
P = 128                  # SBUF partitions
L = T // NCORES          # steps per core = 262144
F = L // P               # steps per partition = 2048
SC_SIZES = (1024, 512, 512)
SC = len(SC_SIZES)
NPOS = 19                # position classes with nonzero matrices
BSH = 8.0                # ACT positivity shift
ACT_LO = 9               # position classes >= this use ACT-produced mv
EW = 4 * NT + 1          # out row: NT matrices (4 entries each) + gold

# param row layout (f32 words): [pos' 19*4 | D 4 | VB 19*4 | negVB 19*4]
# pos' = pos + w2w[1] (folded);  D = w2w[0] - w2w[1]
NPAR = 19 * 4 + 4 + 19 * 4 + 19 * 4
COL_POS = 0
COL_W = 76
COL_VB = 80
COL_NVB = 156

W0 = 2 * NPAR + 5 * F    # fp16 blob columns


_NC_CACHE = None
LAST_RESULTS = None  # BassKernelResults of the most recent kernel() call


def _comp(i, j):
    return i * 2 + j


def _build_nc():
    nc = bacc.Bacc()

    b0_d = nc.dram_tensor("blob0", [P, W0], dt.float16, kind="ExternalInput")
    out_d = nc.dram_tensor("out", [P, EW], dt.float32, kind="ExternalOutput")

    with ExitStack() as ctx:
        tc = ctx.enter_context(tile.TileContext(nc))
        pool = ctx.enter_context(tc.tile_pool(name="main", bufs=1))

        # ---- loads: [par | p | w | msel | em0 | em1] in 3 DMAs ----
        b0 = pool.tile([P, W0], dt.float16, tag="b0", name="b0")
        hq = 2 * NPAR + 1024       # par + p columns for sub-chunk 0
        h0 = 2 * NPAR + F          # par + p
        h1 = h0 + 2 * F            # + w + msel
        nc.sync.dma_start(b0[:, 0:hq], b0_d[:, 0:hq])
        nc.sync.dma_start(b0[:, hq:h0], b0_d[:, hq:h0])
        nc.sync.dma_start(b0[:, h0:h1], b0_d[:, h0:h1])
        nc.sync.dma_start(b0[:, h1:W0], b0_d[:, h1:W0])

        par32 = b0[:, 0:2 * NPAR].bitcast(dt.float32)
        p_t = b0[:, 2 * NPAR:h0]
        w_t = b0[:, h0:h0 + F]
        msel_t = b0[:, h0 + F:h1]
        em0_t = b0[:, h1:h1 + F]
        em1_t = b0[:, h1 + F:W0]

        def V(col):
            return par32[:, col:col + 1]

        bias_c = {}
        for c in range(ACT_LO, NPOS):
            t_ = pool.tile([P, 1], dt.float32, tag=f"bc{c}", name=f"bc{c}")
            nc.vector.memset(t_[:], -float(c))
            bias_c[c] = t_
        bconst = pool.tile([P, 1], dt.float32, tag="bconst", name="bconst")
        nc.vector.memset(bconst[:], BSH)

        FSMAX = max(SC_SIZES)
        HSMAX = FSMAX // 2
        SC_OFF = [sum(SC_SIZES[:i]) for i in range(SC)]
        acc = [
            pool.tile([P, F], dt.float16, tag=f"acc{c}", name=f"acc{c}")
            for c in range(4)
        ]
        # mv work tiles, full-F, sliced per sub-chunk (cross-SC skew spaces
        # same-SC reuse)
        mv = [
            pool.tile([P, F], dt.float16, tag=f"mv{i}", name=f"mv{i}")
            for i in range(3 * 4)
        ]

        def mv_t(ki, c, s):
            return mv[ki * 4 + c][:, SC_OFF[s]:SC_OFF[s] + SC_SIZES[s]]

        amv = [
            pool.tile([P, F], dt.float16, tag=f"amv{i}", name=f"amv{i}")
            for i in range(3 * 4)
        ]

        def amv_t(ki, c, s):
            return amv[ki * 4 + c][:, SC_OFF[s]:SC_OFF[s] + SC_SIZES[s]]

        sqt = [
            pool.tile([P, F], dt.float16, tag=f"sq{i}", name=f"sq{i}")
            for i in range(2)
        ]
        mvb = pool.tile([P, FSMAX], dt.float16, tag="mvb", name="mvb")
        qm = [
            pool.tile([P, FSMAX], dt.float16, tag=f"qm{i}", name=f"qm{i}")
            for i in range(2)
        ]
        gprod = [
            pool.tile([P, FSMAX], dt.float16, tag=f"gp{i}", name=f"gp{i}")
            for i in range(2)
        ]
        gcell = pool.tile([P, 4 * NT], dt.float32, tag="gcell", name="gcell")
        res = pool.tile([P, EW], dt.float32, tag="res", name="res")

        # striped DVE/Pool assignment for accumulate adds
        POOL_NUM, POOL_DEN = 1, 3
        add_ctr = [0] * 4

        def add_eng(comp):
            add_ctr[comp] += 1
            k = (add_ctr[comp] + comp) % POOL_DEN
            return nc.gpsimd if k < POOL_NUM else nc.vector

        def tadd(comp, out, a, b):
            add_eng(comp).tensor_add(out, a, b)

        def sc_views(s, h=0, nh=1):
            sz = SC_SIZES[s] // nh
            lo = SC_OFF[s] + h * sz
            sl = slice(lo, lo + sz)
            return (p_t[:, sl], w_t[:, sl], msel_t[:, sl],
                    [a[:, sl] for a in acc], sl)

        dve_classes = list(range(0, ACT_LO))
        dgroups = [dve_classes[i:i + 3]
                   for i in range(0, len(dve_classes), 3)]
        act_classes = list(range(ACT_LO, NPOS))
        agroups = [act_classes[i:i + 3]
                   for i in range(0, len(act_classes), 3)]

        def emit_dve_group(s, gi):
            grp = dgroups[gi]
            p_s, w_s, m_s, acc_s, sl = sc_views(s)
            FS = SC_SIZES[s]
            for c in range(4):
                tiles = []
                for ki, k in enumerate(grp):
                    m = mv_t(ki, c, s)
                    nc.vector.tensor_scalar(
                        m, p_s, float(k), V(COL_POS + 4 * k + c),
                        ALU.is_equal, ALU.mult,
                    )
                    tiles.append(m)
                if len(tiles) >= 3:
                    tadd(c, tiles[0], tiles[0], tiles[1])
                    rest = tiles[2]
                    if gi == 0:
                        tadd(c, acc_s[c], tiles[0], rest)
                    else:
                        tadd(c, tiles[0], tiles[0], rest)
                        tadd(c, acc_s[c], acc_s[c], tiles[0])
                else:
                    while len(tiles) > 1:
                        tadd(c, tiles[0], tiles[0], tiles[1])
                        tiles = [tiles[0]] + tiles[2:]
                    if gi == 0:
                        nc.vector.tensor_copy(acc_s[c], tiles[0])
                    else:
                        tadd(c, acc_s[c], acc_s[c], tiles[0])

        def emit_act_group(s, gi):
            grp = agroups[gi]
            p_s, w_s, m_s, acc_s, sl = sc_views(s)
            FS = SC_SIZES[s]
            for ki, k in enumerate(grp):
                sq = sqt[ki % 2][:, SC_OFF[s]:SC_OFF[s] + FS]
                nc.scalar.activation(sq, p_s, AF.Square, bias=bias_c[k][:])
                for c in range(4):
                    nc.scalar.activation(
                        amv_t(ki, c, s), sq, AF.Relu,
                        bias=V(COL_VB + 4 * k + c),
                        scale=V(COL_NVB + 4 * k + c),
                    )
            for c in range(4):
                tiles = [amv_t(ki, c, s) for ki in range(len(grp))]
                while len(tiles) > 1:
                    tadd(c, tiles[0], tiles[0], tiles[1])
                    tiles = [tiles[0]] + tiles[2:]
                tadd(c, acc_s[c], acc_s[c], tiles[0])

        def emit_sub_w_em(s, h=0, nh=1):
            p_s, w_s, m_s, acc_s, sl = sc_views(s, h, nh)
            FS = SC_SIZES[s] // nh
            lo = h * FS
            # remove the spurious +B over [p >= ACT_LO] (includes the t=0
            # pad step p=19; host adds B back to both outputs)
            nc.vector.tensor_scalar(
                mvb[:, lo:lo + FS], p_s, ACT_LO - 0.5, bconst[:], ALU.is_gt,
                ALU.mult,
            )
            for c in range(4):
                add_eng(c).tensor_sub(acc_s[c], acc_s[c], mvb[:, lo:lo + FS])
            # who2who: w2w[1] is folded into the position table, so only
            # (w==0)*(w2w[0]-w2w[1]) remains (w==2 occurs only at t=0)
            for c in range(4):
                m0 = mv[0 * 4 + c][:, sl]
                nc.vector.tensor_scalar(
                    m0, w_s, 0.0, V(COL_W + c), ALU.is_equal, ALU.mult,
                )
                tadd(c, acc_s[c], acc_s[c], m0)
            # emission fold: acc[i,j] += em_j
            for c in range(4):
                em_s = (em0_t if c % 2 == 0 else em1_t)[:, sl]
                tadd(c, acc_s[c], acc_s[c], em_s)

        def emit_gold(s, tidx, h=0, nh=1):
            p_s, w_s, m_s, acc_s, sl = sc_views(s, h, nh)
            FS = SC_SIZES[s] // nh
            lo = h * FS
            for pair in range(4):
                q = qm[pair % 2][:, lo:lo + FS]
                nc.vector.tensor_scalar(q, m_s, float(pair), None,
                                        ALU.is_equal)
                pr = gprod[pair % 2][:, lo:lo + FS]
                nc.vector.tensor_mul(pr, q, acc_s[pair])
                nc.vector.tensor_scalar(
                    pr, pr, 1.0, None, ALU.mult, ALU.add,
                    accum_out=gcell[:, tidx * 4 + pair:tidx * 4 + pair + 1],
                )

        def u2(ap):
            return ap.unsqueeze(2).unsqueeze(3)

        HSMAX = max(SC_SIZES) // 2
        FP16_LEVELS = 4

        def tree_scratch(setid, cap):
            def t(tag, shape, dtp):
                return pool.tile(shape, dtp, tag=f"{tag}_{setid}",
                                 name=f"{tag}_{setid}")
            return {
                "cap": cap,
                "sid": setid,
                "X16": t("X16", [P, cap, 2, 2], dt.float16),
                "Y16a": t("Y16a", [P, cap, 2, 2], dt.float16),
                "Y16b": t("Y16b", [P, cap // 2, 2, 2], dt.float16),
                "X32": t("X32", [P, max(cap // 16, 1), 2, 2], dt.float32),
                "Y0": t("Y0", [P, cap, 2, 2], dt.float32),
                "Y1": t("Y1", [P, cap // 2, 2, 2], dt.float32),
            }

        def emit_trees(specs):
            """Emit one or more same-size trees with level-interleaved
            instruction order so concurrent trees fill each other's
            dependency gaps."""
            sts = []
            for (s, h, nh, tidx, scr) in specs:
                _, _, _, acc_s, sl = sc_views(s, h, nh)
                HS = (SC_SIZES[s] // nh) // 2
                sts.append(dict(acc=acc_s, HS=HS, tidx=tidx, scr=scr))
            # level 1 from the acc streams
            for st in sts:
                scr, HS, acc_s = st["scr"], st["HS"], st["acc"]
                X16 = scr["X16"][:, 0:HS]
                Y16a = scr["Y16a"][:, 0:HS]
                for i in range(2):
                    for j in range(2):
                        add_eng(_comp(i, j)).tensor_add(
                            X16[:, :, i:i + 1, j:j + 1],
                            u2(acc_s[_comp(i, 0)][:, 0::2]),
                            u2(acc_s[_comp(0, j)][:, 1::2]),
                        )
                        add_eng(_comp(i, j)).tensor_add(
                            Y16a[:, :, i:i + 1, j:j + 1],
                            u2(acc_s[_comp(i, 1)][:, 0::2]),
                            u2(acc_s[_comp(1, j)][:, 1::2]),
                        )
            for st in sts:
                scr, HS = st["scr"], st["HS"]
                X16, Y16a, Y0 = (scr["X16"][:, 0:HS], scr["Y16a"][:, 0:HS],
                                 scr["Y0"][:, 0:HS])
                nc.vector.tensor_sub(Y16a[:], Y16a[:], X16[:])
                nc.scalar.activation(Y0[:], Y16a[:], AF.Exp)
                nc.scalar.activation(Y0[:], Y0[:], AF.Ln, bias=1.0)
                mlev = pool.tile([P, scr["cap"], 2, 2], dt.float16,
                                 tag=f"m1_s{scr['sid']}",
                                 name=f"m1_s{scr['sid']}")[:, 0:HS]
                nc.vector.tensor_add(mlev[:], X16[:], Y0[:])
                st["mlev"] = mlev
                st["w"] = HS
                st["lev"] = 1
            while any(st["w"] > 1 for st in sts):
                for st in sts:
                    if st["w"] <= 1:
                        continue
                    scr = st["scr"]
                    mlev, w_cur = st["mlev"], st["w"]
                    w2 = w_cur // 2
                    lev = st["lev"] + 1
                    sh = [P, w2, 2, 2]
                    a_i0 = mlev[:, 0:w_cur:2, :, 0:1].broadcast_to(sh)
                    a_i1 = mlev[:, 0:w_cur:2, :, 1:2].broadcast_to(sh)
                    b_0j = mlev[:, 1:w_cur:2, 0:1, :].broadcast_to(sh)
                    b_1j = mlev[:, 1:w_cur:2, 1:2, :].broadcast_to(sh)
                    sp = (scr["Y0"] if lev % 2 == 1 else scr["Y1"])[:, 0:w2]
                    if lev <= FP16_LEVELS:
                        xv = scr["X16"][:, 0:w2]
                        yv = (scr["Y16a"] if lev % 2 == 1
                              else scr["Y16b"])[:, 0:w2]
                        (add_eng(lev % 4) if w2 >= 64 else nc.vector
                         ).tensor_add(xv, a_i0, b_0j)
                        (add_eng((lev + 2) % 4) if w2 >= 64 else nc.vector
                         ).tensor_add(yv, a_i1, b_1j)
                        nc.vector.tensor_sub(yv, yv, xv)
                        nc.scalar.activation(sp, yv, AF.Exp)
                    else:
                        xv = scr["X32"][:, 0:w2]
                        yv = sp
                        nc.vector.tensor_add(xv, a_i0, b_0j)
                        nc.vector.tensor_add(yv, a_i1, b_1j)
                        nc.vector.tensor_sub(yv, yv, xv)
                        nc.scalar.activation(sp, sp, AF.Exp)
                    nc.scalar.activation(sp, sp, AF.Ln, bias=1.0)
                    mdt = dt.float16 if lev <= FP16_LEVELS else dt.float32
                    mwidth = max(scr["cap"] // (2 ** (lev - 1)), 1)
                    mnext = pool.tile(
                        [P, mwidth, 2, 2], mdt,
                        tag=f"m{lev}_s{scr['sid']}",
                        name=f"m{lev}_s{scr['sid']}")[:, 0:w2]
                    nc.vector.tensor_add(mnext[:], xv, sp)
                    st["mlev"] = mnext
                    st["w"] = w2
                    st["lev"] = lev
            for st in sts:
                nc.vector.tensor_copy(
                    res[:, 4 * st["tidx"]:4 * st["tidx"] + 4].rearrange(
                        "p (a b c) -> p a b c", a=1, b=2),
                    st["mlev"][:],
                )

        scr0 = tree_scratch(0, HSMAX)
        scr1 = tree_scratch(1, HSMAX // 2)

        # ---- skewed block emission: sub-chunk s trails s-1 by SKEW blocks
        # so completions stagger; the LAST sub-chunk's tail (links, gold,
        # tree) is split into two concurrent halves to break the final
        # serial LSE chain ----
        SKEW = 6
        blocks = []
        for s in range(SC):
            seq = []
            seq.append(lambda s=s: emit_dve_group(s, 0))
            for gi in range(len(agroups)):
                seq.append(lambda s=s, gi=gi: emit_act_group(s, gi))
                if gi + 1 < len(dgroups):
                    seq.append(lambda s=s, gi=gi: emit_dve_group(s, gi + 1))
            if s < SC - 1:
                seq.append(lambda s=s: emit_sub_w_em(s))
                seq.append(lambda s=s: emit_gold(s, s))
                seq.append(lambda s=s: emit_trees([(s, 0, 1, s, scr0)]))
            else:
                seq.append(lambda s=s: emit_sub_w_em(s, 0, 2))
                seq.append(lambda s=s: emit_gold(s, s, 0, 2))
                seq.append(lambda s=s: emit_sub_w_em(s, 1, 2))
                seq.append(lambda s=s: emit_gold(s, s + 1, 1, 2))
                seq.append(lambda s=s: emit_trees(
                    [(s, 0, 2, s, scr0), (s, 1, 2, s + 1, scr1)]))
            for bi, fn in enumerate(seq):
                blocks.append((bi + SKEW * s, s, fn))
        blocks.sort(key=lambda kv: (kv[0], kv[1]))
        for _, _, fn in blocks:
            fn()

    nc.compile()

    # Exp/Ln/Square/Relu all live in 'natural_log_exp_and_others', but
    # insert_act_table_loads picks the first set containing each function,
    # emitting alternating table reloads (1.3 us each).  Retarget every load
    # to the combined set and drop the now-redundant ones.
    from concourse.hw_specs import get_activation_tables

    tables = list(get_activation_tables(nc.m.arch).keys())
    combined = tables.index("natural_log_exp_and_others")
    for b in nc.bb_map.values():
        insts = b.bb.instructions
        kept = []
        seen_load = False
        for ins in insts:
            if ins.opcode == "LoadActFuncSet":
                si = ins.sync_info
                assert not (si and (si.on_wait or si.on_update)), ins.name
                if seen_load:
                    continue
                ins.act_func_set_id = combined
                seen_load = True
            kept.append(ins)
        if len(kept) != len(insts):
            b.bb.instructions = kept
    return nc


def _get_nc():
    global _NC_CACHE
    if _NC_CACHE is None:
        _NC_CACHE = _build_nc()
    return _NC_CACHE


def _f16_candidates(x, grid_pow=None):
    """Nearest fp16 (or 2^grid_pow-grid) value and its other-side neighbor."""
    if grid_pow is None:
        lo = np.float16(x)
        res = float(x) - float(lo)
        if res == 0.0:
            return np.float32(lo), np.float32(lo)
        hi = np.nextafter(lo, np.float16(np.inf if res > 0 else -np.inf),
                          dtype=np.float16)
        return np.float32(lo), np.float32(hi)
    g = 2.0 ** grid_pow
    lo = np.floor(float(x) / g) * g
    hi = lo + g
    if abs(float(x) - lo) <= abs(hi - float(x)):
        return np.float32(lo), np.float32(hi)
    return np.float32(hi), np.float32(lo)


def _optimize_tables(pos, w2w):
    """fp16 tables with per-entry rounding direction chosen so the
    systematic selected-sum bias over the reachable (p,w) cells cancels.
    w2w[1] is folded into the position table (P' = pos + w2w[1]); the
    residual class value is D = w2w[0] - w2w[1].  ACT-suffix rows sit on
    the 2^-7 grid so V+BSH stays fp16-exact."""
    posr = pos.reshape(NPOS, 4).astype(np.float64)
    wr = w2w.reshape(2, 4).astype(np.float64)
    Pp = posr + wr[1]
    Dv = wr[0] - wr[1]

    Pc = np.zeros((NPOS, 4, 2), np.float32)
    for k in range(NPOS):
        gp = -7 if k >= ACT_LO else None
        for c in range(4):
            Pc[k, c] = _f16_candidates(Pp[k, c], gp)
    Dc = np.zeros((4, 2), np.float32)
    for c in range(4):
        Dc[c] = _f16_candidates(Dv[c])

    # cell (k, w=1): value = P16[k];  cell (k, w=0): fp16(P16[k] + D16)
    e1 = posr + wr[1]
    e0 = posr + wr[0]
    d1 = Pc.astype(np.float64) - e1[:, :, None]               # [19,4,2]
    d0 = ((Pc[:, :, :, None].astype(np.float16)
           + Dc[None, :, None, :].astype(np.float16)).astype(np.float16)
          .astype(np.float64) - e0[:, :, None, None])          # [19,4,2,2]

    Ps = np.zeros((NPOS, 4), np.intp)
    Ds = np.zeros(4, np.intp)

    def total():
        s = 0.0
        for k in range(NPOS):
            for c in range(4):
                s += d1[k, c, Ps[k, c]] + d0[k, c, Ps[k, c], Ds[c]]
        return s

    best = total()
    for _ in range(4):
        improved = False
        for k in range(NPOS):
            for c in range(4):
                Ps[k, c] ^= 1
                t2 = total()
                if abs(t2) < abs(best):
                    best = t2
                    improved = True
                else:
                    Ps[k, c] ^= 1
        for c in range(4):
            Ds[c] ^= 1
            t2 = total()
            if abs(t2) < abs(best):
                best = t2
                improved = True
            else:
                Ds[c] ^= 1
        if not improved:
            break

    P16 = np.take_along_axis(Pc, Ps[:, :, None], axis=2)[:, :, 0]
    D16 = Dc[np.arange(4), Ds]
    return P16.astype(np.float32), D16.astype(np.float32)


def _lse_combine(A, B):
    """ordered log-semiring 2x2 product, vectorized over leading dims"""
    return np.logaddexp(A[..., :, 0:1] + B[..., 0:1, :],
                        A[..., :, 1:2] + B[..., 1:2, :])


def kernel(**inputs):
    em = np.asarray(inputs["emission_scores"], dtype=np.float32)
    lab = np.asarray(inputs["label"]).astype(np.float32)
    w = np.asarray(inputs["who2who_state"]).astype(np.float32)
    p = np.asarray(inputs["position_state"]).astype(np.float32)
    w2w = np.asarray(inputs["who2who_params"], dtype=np.float32)
    pos = np.asarray(inputs["position_params"], dtype=np.float32)
    assert em.shape == (T, 2), em.shape

    labp = np.empty_like(lab)
    labp[0] = 0.0
    labp[1:] = lab[:-1]
    msel = (2.0 * labp + lab).astype(np.float16)

    P16, D16 = _optimize_tables(pos, w2w)
    vb = P16 + np.float32(BSH)   # fp16-exact for the ACT rows (2^-7 grid)
    par_row = np.concatenate([
        P16.reshape(-1), D16.reshape(-1), vb.reshape(-1), (-vb).reshape(-1)
    ]).astype(np.float32)
    assert par_row.shape[0] == NPAR
    par16 = np.broadcast_to(par_row.view(np.float16), (P, 2 * NPAR))

    p16 = p.astype(np.float16)
    w16 = w.astype(np.float16)
    em16 = em.astype(np.float16)

    in_maps = []
    for k in range(NCORES):
        sl = slice(k * L, (k + 1) * L)
        blob0 = np.concatenate(
            [
                par16,
                p16[sl].reshape(P, F),
                w16[sl].reshape(P, F),
                msel[sl].reshape(P, F),
                np.ascontiguousarray(em16[sl, 0].reshape(P, F)),
                np.ascontiguousarray(em16[sl, 1].reshape(P, F)),
            ],
            axis=1,
        )
        in_maps.append({"blob0": np.ascontiguousarray(blob0)})

    nc = _get_nc()
    kr = bass_utils.run_bass_kernel_spmd(nc, in_maps, core_ids=list(range(NCORES)))
    global LAST_RESULTS
    LAST_RESULTS = kr
    results = kr.results

    # host combine: ordered product of NCORES*P*SC 2x2 matrices + gold sum
    rows = np.stack([np.asarray(r["out"], dtype=np.float64) for r in results])
    gold = rows[:, :, 4 * NT].sum()
    mats = rows[:, :, 0:4 * NT].reshape(NCORES * P * NT, 2, 2)
    # pairwise tree keeps it fast and stable
    while mats.shape[0] > 1:
        n = mats.shape[0]
        even = mats[0:n - 1:2]
        odd = mats[1:n:2]
        comb = _lse_combine(even, odd)
        if n % 2 == 1:
            comb = np.concatenate([comb, mats[n - 1:n]], axis=0)
        mats = comb
    total = np.logaddexp.reduce(mats.reshape(-1))
    # the single t=0 pad step (p=19) carries the -BSH shift: add it back
    gold += BSH
    total += BSH
    return np.stack([gold, total]).astype(np.float32)


if __name__ == "__main__":
    rng = np.random.default_rng(0)
    demo = dict(
        emission_scores=rng.standard_normal((T, 2)).astype(np.float32),
        label=rng.integers(0, 2, T),
        who2who_state=np.concatenate([[2], rng.integers(0, 2, T - 1)]),
        position_state=np.concatenate([[19], rng.integers(0, 19, T - 1)]),
        who2who_params=rng.standard_normal((2, 2, 2)).astype(np.float32),
        position_params=rng.standard_normal((19, 2, 2)).astype(np.float32),
    )
    print(kernel(**demo))


# revision 30
# speedup vs baseline: 1.0169x; 1.0007x over previous
"""Trainium2 Bass kernel for a 2-state linear-chain CRF loss (BiLSTM-CRF loss_fn).

Computes, for a single conversation of length T = 2,097,152:
  gold_score  = sum_t em[t, lab[t]] + sum_{t>0} trans[t][lab[t-1], lab[t]]
  total_score = logsumexp of the CRF forward recursion
where trans[t] = who2who_sub[w[t]] + position_sub[p[t]] (60 possible 2x2
matrices; indices 2/19 select an all-zero padding matrix).

Design (one NeuronCore per contiguous chunk of 262,144 steps, 8 cores):

* Per-step matrices: trans+em is built as 4 fp16 streams by per-class masked
  accumulation (19 position classes + 2 who2who classes + emission fold).
  Class supports are disjoint, so sums of masked values are exact in fp16;
  the masked values are combined PAIRWISE (a small in-group tree) so the
  per-stream dependency depth is ~8 instead of 21 serial adds.  Work is
  split three ways: DVE runs fused (idx==c)*V tensor_scalars (4x fp16 mode)
  plus most adds; the ACT engine produces masked values for a suffix of
  position classes as Relu((V+B) - (V+B)*(p-c)^2) with B=4 making the peak
  positive (the spurious +B*[p>=a] is removed by one (p>a-.5)*B mask and 4
  subtracts; the t=0 pad step ends shifted by exactly -B, corrected on the
  host); GPSIMD takes a striped share of the adds.

* Gold score: the label-pair stream msel = 2*lab[t-1]+lab[t] selects one of
  the 4 finished streams per step; gold = sum_t acc[msel_t][t] via 4
  is_equal masks + multiply + accum_out per sub-chunk.  Exactness: stream
  values are single-fp16-rounded table values (+ exact-in-fp16 shifts), and
  the host chooses each table entry's fp16 rounding DIRECTION (greedy sign
  optimization over the 19x2x4 reachable cells) so the systematic selection
  bias cancels to ~1e-5 relative.

* Forward pass: the recursion is a product of 2x2 matrices in the (log, +)
  semiring; each core tree-reduces with LSE(a,b) = a + ln(1+exp(b-a)) on
  ACT.  The chunk is split into 3 sub-chunks of 1024/512/512 steps per
  partition so each sub-chunk's tree overlaps the next one's stream build
  and only the last (small) tree is exposed at the end.  Each core ships
  its 3*128 sub-chunk matrices + per-partition gold; the host does the
  O(cores*P) ordered log-semiring combine (vectorized numpy).

* All inputs ship as one fp16 blob [par | p | w | msel | em0 | em1] in 3
  DMAs so the class masks start immediately.
"""

from contextlib import ExitStack

import numpy as np

import concourse.bass as bass
import concourse.bacc as bacc
import concourse.mybir as mybir
import concourse.tile as tile
from concourse import bass_utils

dt = mybir.dt
ALU = mybir.AluOpType
AF = mybir.ActivationFunctionType
AX = mybir.AxisListType

T = 2097152
NCORES = 8
P = 128                  # SBUF partitions
L = T // NCORES          # steps per core = 262144
F = L // P               # steps per partition = 2048
SC_SIZES = (1024, 512, 512)
SC = len(SC_SIZES)
NPOS = 19                # position classes with nonzero matrices
BSH = 8.0                # ACT positivity shift
ACT_LO = 9               # position classes >= this use ACT-produced mv
EW = 4 * NT + 1          # out row: NT matrices (4 entries each) + gold

# param row layout (f32 words): [pos' 19*4 | D 4 | VB 19*4 | negVB 19*4]
# pos' = pos + w2w[1] (folded);  D = w2w[0] - w2w[1]
NPAR = 19 * 4 + 4 + 19 * 4 + 19 * 4
COL_POS = 0
COL_W = 76
COL_VB = 80
COL_NVB = 156

W0 = 2 * NPAR + 5 * F    # fp16 blob columns


_NC_CACHE = None
LAST_RESULTS = None  # BassKernelResults of the most recent kernel() call


def _comp(i, j):
    return i * 2 + j


def _build_nc():
    nc = bacc.Bacc()

    b0_d = nc.dram_tensor("blob0", [P, W0], dt.float16, kind="ExternalInput")
    out_d = nc.dram_tensor("out", [P, EW], dt.float32, kind="ExternalOutput")

    with ExitStack() as ctx:
        tc = ctx.enter_context(tile.TileContext(nc))
        pool = ctx.enter_context(tc.tile_pool(name="main", bufs=1))

        # ---- loads: [par | p | w | msel | em0 | em1] in 3 DMAs ----
        b0 = pool.tile([P, W0], dt.float16, tag="b0", name="b0")
        hq = 2 * NPAR + 1024       # par + p columns for sub-chunk 0
        h0 = 2 * NPAR + F          # par + p
        h1 = h0 + 2 * F            # + w + msel
        nc.sync.dma_start(b0[:, 0:hq], b0_d[:, 0:hq])
        nc.sync.dma_start(b0[:, hq:h0], b0_d[:, hq:h0])
        nc.sync.dma_start(b0[:, h0:h1], b0_d[:, h0:h1])
        nc.sync.dma_start(b0[:, h1:W0], b0_d[:, h1:W0])

        par32 = b0[:, 0:2 * NPAR].bitcast(dt.float32)
        p_t = b0[:, 2 * NPAR:h0]
        w_t = b0[:, h0:h0 + F]
        msel_t = b0[:, h0 + F:h1]
        em0_t = b0[:, h1:h1 + F]
        em1_t = b0[:, h1 + F:W0]

        def V(col):
            return par32[:, col:col + 1]

        bias_c = {}
        for c in range(ACT_LO, NPOS):
            t_ = pool.tile([P, 1], dt.float32, tag=f"bc{c}", name=f"bc{c}")
            nc.vector.memset(t_[:], -float(c))
            bias_c[c] = t_
        bconst = pool.tile([P, 1], dt.float32, tag="bconst", name="bconst")
        nc.vector.memset(bconst[:], BSH)

        FSMAX = max(SC_SIZES)
        HSMAX = FSMAX // 2
        SC_OFF = [sum(SC_SIZES[:i]) for i in range(SC)]
        acc = [
            pool.tile([P, F], dt.float16, tag=f"acc{c}", name=f"acc{c}")
            for c in range(4)
        ]
        # mv work tiles, full-F, sliced per sub-chunk (cross-SC skew spaces
        # same-SC reuse)
        mv = [
            pool.tile([P, F], dt.float16, tag=f"mv{i}", name=f"mv{i}")
            for i in range(3 * 4)
        ]

        def mv_t(ki, c, s):
            return mv[ki * 4 + c][:, SC_OFF[s]:SC_OFF[s] + SC_SIZES[s]]

        amv = [
            pool.tile([P, F], dt.float16, tag=f"amv{i}", name=f"amv{i}")
            for i in range(3 * 4)
        ]

        def amv_t(ki, c, s):
            return amv[ki * 4 + c][:, SC_OFF[s]:SC_OFF[s] + SC_SIZES[s]]

        sqt = [
            pool.tile([P, F], dt.float16, tag=f"sq{i}", name=f"sq{i}")
            for i in range(2)
        ]
        mvb = pool.tile([P, FSMAX], dt.float16, tag="mvb", name="mvb")
        qm = [
            pool.tile([P, FSMAX], dt.float16, tag=f"qm{i}", name=f"qm{i}")
            for i in range(2)
        ]
        gprod = [
            pool.tile([P, FSMAX], dt.float16, tag=f"gp{i}", name=f"gp{i}")
            for i in range(2)
        ]
        gcell = pool.tile([P, 4 * NT], dt.float32, tag="gcell", name="gcell")
        res = pool.tile([P, EW], dt.float32, tag="res", name="res")

        # striped DVE/Pool assignment for accumulate adds
        POOL_NUM, POOL_DEN = 1, 3
        add_ctr = [0] * 4

        def add_eng(comp):
            add_ctr[comp] += 1
            k = (add_ctr[comp] * 3 + comp) % POOL_DEN
            return nc.gpsimd if k < POOL_NUM else nc.vector

        def tadd(comp, out, a, b):
            add_eng(comp).tensor_add(out, a, b)

        def sc_views(s, h=0, nh=1):
            sz = SC_SIZES[s] // nh
            lo = SC_OFF[s] + h * sz
            sl = slice(lo, lo + sz)
            return (p_t[:, sl], w_t[:, sl], msel_t[:, sl],
                    [a[:, sl] for a in acc], sl)

        dve_classes = list(range(0, ACT_LO))
        dgroups = [dve_classes[i:i + 3]
                   for i in range(0, len(dve_classes), 3)]
        act_classes = list(range(ACT_LO, NPOS))
        agroups = [act_classes[i:i + 3]
                   for i in range(0, len(act_classes), 3)]

        def emit_dve_group(s, gi):
            grp = dgroups[gi]
            p_s, w_s, m_s, acc_s, sl = sc_views(s)
            FS = SC_SIZES[s]
            for c in range(4):
                tiles = []
                for ki, k in enumerate(grp):
                    m = mv_t(ki, c, s)
                    nc.vector.tensor_scalar(
                        m, p_s, float(k), V(COL_POS + 4 * k + c),
                        ALU.is_equal, ALU.mult,
                    )
                    tiles.append(m)
                if len(tiles) >= 3:
                    tadd(c, tiles[0], tiles[0], tiles[1])
                    rest = tiles[2]
                    if gi == 0:
                        tadd(c, acc_s[c], tiles[0], rest)
                    else:
                        tadd(c, tiles[0], tiles[0], rest)
                        tadd(c, acc_s[c], acc_s[c], tiles[0])
                else:
                    while len(tiles) > 1:
                        tadd(c, tiles[0], tiles[0], tiles[1])
                        tiles = [tiles[0]] + tiles[2:]
                    if gi == 0:
                        nc.vector.tensor_copy(acc_s[c], tiles[0])
                    else:
                        tadd(c, acc_s[c], acc_s[c], tiles[0])

        def emit_act_group(s, gi):
            grp = agroups[gi]
            p_s, w_s, m_s, acc_s, sl = sc_views(s)
            FS = SC_SIZES[s]
            for ki, k in enumerate(grp):
                sq = sqt[ki % 2][:, SC_OFF[s]:SC_OFF[s] + FS]
                nc.scalar.activation(sq, p_s, AF.Square, bias=bias_c[k][:])
                for c in range(4):
                    nc.scalar.activation(
                        amv_t(ki, c, s), sq, AF.Relu,
                        bias=V(COL_VB + 4 * k + c),
                        scale=V(COL_NVB + 4 * k + c),
                    )
            for c in range(4):
                tiles = [amv_t(ki, c, s) for ki in range(len(grp))]
                while len(tiles) > 1:
                    tadd(c, tiles[0], tiles[0], tiles[1])
                    tiles = [tiles[0]] + tiles[2:]
                tadd(c, acc_s[c], acc_s[c], tiles[0])

        def emit_sub_w_em(s, h=0, nh=1):
            p_s, w_s, m_s, acc_s, sl = sc_views(s, h, nh)
            FS = SC_SIZES[s] // nh
            lo = h * FS
            # remove the spurious +B over [p >= ACT_LO] (includes the t=0
            # pad step p=19; host adds B back to both outputs)
            nc.vector.tensor_scalar(
                mvb[:, lo:lo + FS], p_s, ACT_LO - 0.5, bconst[:], ALU.is_gt,
                ALU.mult,
            )
            for c in range(4):
                add_eng(c).tensor_sub(acc_s[c], acc_s[c], mvb[:, lo:lo + FS])
            # who2who: w2w[1] is folded into the position table, so only
            # (w==0)*(w2w[0]-w2w[1]) remains (w==2 occurs only at t=0)
            for c in range(4):
                m0 = mv[0 * 4 + c][:, sl]
                nc.vector.tensor_scalar(
                    m0, w_s, 0.0, V(COL_W + c), ALU.is_equal, ALU.mult,
                )
                tadd(c, acc_s[c], acc_s[c], m0)
            # emission fold: acc[i,j] += em_j
            for c in range(4):
                em_s = (em0_t if c % 2 == 0 else em1_t)[:, sl]
                tadd(c, acc_s[c], acc_s[c], em_s)

        def emit_gold(s, tidx, h=0, nh=1):
            p_s, w_s, m_s, acc_s, sl = sc_views(s, h, nh)
            FS = SC_SIZES[s] // nh
            lo = h * FS
            for pair in range(4):
                q = qm[pair % 2][:, lo:lo + FS]
                nc.vector.tensor_scalar(q, m_s, float(pair), None,
                                        ALU.is_equal)
                pr = gprod[pair % 2][:, lo:lo + FS]
                nc.vector.tensor_mul(pr, q, acc_s[pair])
                nc.vector.tensor_scalar(
                    pr, pr, 1.0, None, ALU.mult, ALU.add,
                    accum_out=gcell[:, tidx * 4 + pair:tidx * 4 + pair + 1],
                )

        def u2(ap):
            return ap.unsqueeze(2).unsqueeze(3)

        HSMAX = max(SC_SIZES) // 2
        FP16_LEVELS = 4

        def tree_scratch(setid, cap):
            def t(tag, shape, dtp):
                return pool.tile(shape, dtp, tag=f"{tag}_{setid}",
                                 name=f"{tag}_{setid}")
            return {
                "cap": cap,
                "sid": setid,
                "X16": t("X16", [P, cap, 2, 2], dt.float16),
                "Y16a": t("Y16a", [P, cap, 2, 2], dt.float16),
                "Y16b": t("Y16b", [P, cap // 2, 2, 2], dt.float16),
                "X32": t("X32", [P, max(cap // 16, 1), 2, 2], dt.float32),
                "Y0": t("Y0", [P, cap, 2, 2], dt.float32),
                "Y1": t("Y1", [P, cap // 2, 2, 2], dt.float32),
            }

        def emit_trees(specs):
            """Emit one or more same-size trees with level-interleaved
            instruction order so concurrent trees fill each other's
            dependency gaps."""
            sts = []
            for (s, h, nh, tidx, scr) in specs:
                _, _, _, acc_s, sl = sc_views(s, h, nh)
                HS = (SC_SIZES[s] // nh) // 2
                sts.append(dict(acc=acc_s, HS=HS, tidx=tidx, scr=scr))
            # level 1 from the acc streams
            for st in sts:
                scr, HS, acc_s = st["scr"], st["HS"], st["acc"]
                X16 = scr["X16"][:, 0:HS]
                Y16a = scr["Y16a"][:, 0:HS]
                for i in range(2):
                    for j in range(2):
                        add_eng(_comp(i, j)).tensor_add(
                            X16[:, :, i:i + 1, j:j + 1],
                            u2(acc_s[_comp(i, 0)][:, 0::2]),
                            u2(acc_s[_comp(0, j)][:, 1::2]),
                        )
                        add_eng(_comp(i, j)).tensor_add(
                            Y16a[:, :, i:i + 1, j:j + 1],
                            u2(acc_s[_comp(i, 1)][:, 0::2]),
                            u2(acc_s[_comp(1, j)][:, 1::2]),
                        )
            for st in sts:
                scr, HS = st["scr"], st["HS"]
                X16, Y16a, Y0 = (scr["X16"][:, 0:HS], scr["Y16a"][:, 0:HS],
                                 scr["Y0"][:, 0:HS])
                nc.vector.tensor_sub(Y16a[:], Y16a[:], X16[:])
                nc.scalar.activation(Y0[:], Y16a[:], AF.Exp)
                nc.scalar.activation(Y0[:], Y0[:], AF.Ln, bias=1.0)
                mlev = pool.tile([P, scr["cap"], 2, 2], dt.float16,
                                 tag=f"m1_s{scr['sid']}",
                                 name=f"m1_s{scr['sid']}")[:, 0:HS]
                nc.vector.tensor_add(mlev[:], X16[:], Y0[:])
                st["mlev"] = mlev
                st["w"] = HS
                st["lev"] = 1
            while any(st["w"] > 1 for st in sts):
                for st in sts:
                    if st["w"] <= 1:
                        continue
                    scr = st["scr"]
                    mlev, w_cur = st["mlev"], st["w"]
                    w2 = w_cur // 2
                    lev = st["lev"] + 1
                    sh = [P, w2, 2, 2]
                    a_i0 = mlev[:, 0:w_cur:2, :, 0:1].broadcast_to(sh)
                    a_i1 = mlev[:, 0:w_cur:2, :, 1:2].broadcast_to(sh)
                    b_0j = mlev[:, 1:w_cur:2, 0:1, :].broadcast_to(sh)
                    b_1j = mlev[:, 1:w_cur:2, 1:2, :].broadcast_to(sh)
                    sp = (scr["Y0"] if lev % 2 == 1 else scr["Y1"])[:, 0:w2]
                    if lev <= FP16_LEVELS:
                        xv = scr["X16"][:, 0:w2]
                        yv = (scr["Y16a"] if lev % 2 == 1
                              else scr["Y16b"])[:, 0:w2]
                        (add_eng(lev % 4) if w2 >= 64 else nc.vector
                         ).tensor_add(xv, a_i0, b_0j)
                        (add_eng((lev + 2) % 4) if w2 >= 64 else nc.vector
                         ).tensor_add(yv, a_i1, b_1j)
                        nc.vector.tensor_sub(yv, yv, xv)
                        nc.scalar.activation(sp, yv, AF.Exp)
                    else:
                        xv = scr["X32"][:, 0:w2]
                        yv = sp
                        nc.vector.tensor_add(xv, a_i0, b_0j)
                        nc.vector.tensor_add(yv, a_i1, b_1j)
                        nc.vector.tensor_sub(yv, yv, xv)
                        nc.scalar.activation(sp, sp, AF.Exp)
                    nc.scalar.activation(sp, sp, AF.Ln, bias=1.0)
                    mdt = dt.float16 if lev <= FP16_LEVELS else dt.float32
                    mwidth = max(scr["cap"] // (2 ** (lev - 1)), 1)
                    mnext = pool.tile(
                        [P, mwidth, 2, 2], mdt,
                        tag=f"m{lev}_s{scr['sid']}",
                        name=f"m{lev}_s{scr['sid']}")[:, 0:w2]
                    nc.vector.tensor_add(mnext[:], xv, sp)
                    st["mlev"] = mnext
                    st["w"] = w2
                    st["lev"] = lev
            for st in sts:
                nc.vector.tensor_copy(
                    res[:, 4 * st["tidx"]:4 * st["tidx"] + 4].rearrange(
                        "p (a b c) -> p a b c", a=1, b=2),
                    st["mlev"][:],
                )

        scr0 = tree_scratch(0, HSMAX)
        scr1 = tree_scratch(1, HSMAX // 2)

        # ---- skewed block emission: sub-chunk s trails s-1 by SKEW blocks
        # so completions stagger; the LAST sub-chunk's tail (links, gold,
        # tree) is split into two concurrent halves to break the final
        # serial LSE chain ----
        SKEW = 6
        blocks = []
        for s in range(SC):
            seq = []
            seq.append(lambda s=s: emit_dve_group(s, 0))
            for gi in range(len(agroups)):
                seq.append(lambda s=s, gi=gi: emit_act_group(s, gi))
                if gi + 1 < len(dgroups):
                    seq.append(lambda s=s, gi=gi: emit_dve_group(s, gi + 1))
            if s < SC - 1:
                seq.append(lambda s=s: emit_sub_w_em(s))
                seq.append(lambda s=s: emit_gold(s, s))
                seq.append(lambda s=s: emit_trees([(s, 0, 1, s, scr0)]))
            else:
                seq.append(lambda s=s: emit_sub_w_em(s, 0, 2))
                seq.append(lambda s=s: emit_gold(s, s, 0, 2))
                seq.append(lambda s=s: emit_sub_w_em(s, 1, 2))
                seq.append(lambda s=s: emit_gold(s, s + 1, 1, 2))
                seq.append(lambda s=s: emit_trees(
                    [(s, 0, 2, s, scr0), (s, 1, 2, s + 1, scr1)]))
            for bi, fn in enumerate(seq):
                blocks.append((bi + SKEW * s, s, fn))
        blocks.sort(key=lambda kv: (kv[0], kv[1]))
        for _, _, fn in blocks:
            fn()

    nc.compile()

    # Exp/Ln/Square/Relu all live in 'natural_log_exp_and_others', but
    # insert_act_table_loads picks the first set containing each function,
    # emitting alternating table reloads (1.3 us each).  Retarget every load
    # to the combined set and drop the now-redundant ones.
    from concourse.hw_specs import get_activation_tables

    tables = list(get_activation_tables(nc.m.arch).keys())
    combined = tables.index("natural_log_exp_and_others")
    for b in nc.bb_map.values():
        insts = b.bb.instructions
        kept = []
        seen_load = False
        for ins in insts:
            if ins.opcode == "LoadActFuncSet":
                si = ins.sync_info
                assert not (si and (si.on_wait or si.on_update)), ins.name
                if seen_load:
                    continue
                ins.act_func_set_id = combined
                seen_load = True
            kept.append(ins)
        if len(kept) != len(insts):
            b.bb.instructions = kept
    return nc


def _get_nc():
    global _NC_CACHE
    if _NC_CACHE is None:
        _NC_CACHE = _build_nc()
    return _NC_CACHE


def _f16_candidates(x, grid_pow=None):
    """Nearest fp16 (or 2^grid_pow-grid) value and its other-side neighbor."""
    if grid_pow is None:
        lo = np.float16(x)
        res = float(x) - float(lo)
        if res == 0.0:
            return np.float32(lo), np.float32(lo)
        hi = np.nextafter(lo, np.float16(np.inf if res > 0 else -np.inf),
                          dtype=np.float16)
        return np.float32(lo), np.float32(hi)
    g = 2.0 ** grid_pow
    lo = np.floor(float(x) / g) * g
    hi = lo + g
    if abs(float(x) - lo) <= abs(hi - float(x)):
        return np.float32(lo), np.float32(hi)
    return np.float32(hi), np.float32(lo)


def _optimize_tables(pos, w2w):
    """fp16 tables with per-entry rounding direction chosen so the
    systematic selected-sum bias over the reachable (p,w) cells cancels.
    w2w[1] is folded into the position table (P' = pos + w2w[1]); the
    residual class value is D = w2w[0] - w2w[1].  ACT-suffix rows sit on
    the 2^-7 grid so V+BSH stays fp16-exact."""
    posr = pos.reshape(NPOS, 4).astype(np.float64)
    wr = w2w.reshape(2, 4).astype(np.float64)
    Pp = posr + wr[1]
    Dv = wr[0] - wr[1]

    Pc = np.zeros((NPOS, 4, 2), np.float32)
    for k in range(NPOS):
        gp = -7 if k >= ACT_LO else None
        for c in range(4):
            Pc[k, c] = _f16_candidates(Pp[k, c], gp)
    Dc = np.zeros((4, 2), np.float32)
    for c in range(4):
        Dc[c] = _f16_candidates(Dv[c])

    # cell (k, w=1): value = P16[k];  cell (k, w=0): fp16(P16[k] + D16)
    e1 = posr + wr[1]
    e0 = posr + wr[0]
    d1 = Pc.astype(np.float64) - e1[:, :, None]               # [19,4,2]
    d0 = ((Pc[:, :, :, None].astype(np.float16)
           + Dc[None, :, None, :].astype(np.float16)).astype(np.float16)
          .astype(np.float64) - e0[:, :, None, None])          # [19,4,2,2]

    Ps = np.zeros((NPOS, 4), np.intp)
    Ds = np.zeros(4, np.intp)

    def total():
        s = 0.0
        for k in range(NPOS):
            for c in range(4):
                s += d1[k, c, Ps[k, c]] + d0[k, c, Ps[k, c], Ds[c]]
        return s

    best = total()
    for _ in range(4):
        improved = False
        for k in range(NPOS):
            for c in range(4):
                Ps[k, c] ^= 1
                t2 = total()
                if abs(t2) < abs(best):
                    best = t2
                    improved = True
                else:
                    Ps[k, c] ^= 1
        for c in range(4):
            Ds[c] ^= 1
            t2 = total()
            if abs(t2) < abs(best):
                best = t2
                improved = True
            else:
                Ds[c] ^= 1
        if not improved:
            break

    P16 = np.take_along_axis(Pc, Ps[:, :, None], axis=2)[:, :, 0]
    D16 = Dc[np.arange(4), Ds]
    return P16.astype(np.float32), D16.astype(np.float32)


def _lse_combine(A, B):
    """ordered log-semiring 2x2 product, vectorized over leading dims"""
    return np.logaddexp(A[..., :, 0:1] + B[..., 0:1, :],
                        A[..., :, 1:2] + B[..., 1:2, :])


def kernel(**inputs):
    em = np.asarray(inputs["emission_scores"], dtype=np.float32)
    lab = np.asarray(inputs["label"]).astype(np.float32)
    w = np.asarray(inputs["who2who_state"]).astype(np.float32)
    p = np.asarray(inputs["position_state"]).astype(np.float32)
    w2w = np.asarray(inputs["who2who_params"], dtype=np.float32)
    pos = np.asarray(inputs["position_params"], dtype=np.float32)
    assert em.shape == (T, 2), em.shape

    labp = np.empty_like(lab)
    labp[0] = 0.0
    labp[1:] = lab[:-1]
    msel = (2.0 * labp + lab).astype(np.float16)

    P16, D16 = _optimize_tables(pos, w2w)
    vb = P16 + np.float32(BSH)   # fp16-exact for the ACT rows (2^-7 grid)
    par_row = np.concatenate([
        P16.reshape(-1), D16.reshape(-1), vb.reshape(-1), (-vb).reshape(-1)
    ]).astype(np.float32)
    assert par_row.shape[0] == NPAR
    par16 = np.broadcast_to(par_row.view(np.float16), (P, 2 * NPAR))

    p16 = p.astype(np.float16)
    w16 = w.astype(np.float16)
    em16 = em.astype(np.float16)

    in_maps = []
    for k in range(NCORES):
        sl = slice(k * L, (k + 1) * L)
        blob0 = np.concatenate(
            [
                par16,
                p16[sl].reshape(P, F),
                w16[sl].reshape(P, F),
                msel[sl].reshape(P, F),
                np.ascontiguousarray(em16[sl, 0].reshape(P, F)),
                np.ascontiguousarray(em16[sl, 1].reshape(P, F)),
            ],
            axis=1,
        )
        in_maps.append({"blob0": np.ascontiguousarray(blob0)})

    nc = _get_nc()
    kr = bass_utils.run_bass_kernel_spmd(nc, in_maps, core_ids=list(range(NCORES)))
    global LAST_RESULTS
    LAST_RESULTS = kr
    results = kr.results

    # host combine: ordered product of NCORES*P*SC 2x2 matrices + gold sum
    rows = np.stack([np.asarray(r["out"], dtype=np.float64) for r in results])
    gold = rows[:, :, 4 * NT].sum()
    mats = rows[:, :, 0:4 * NT].reshape(NCORES * P * NT, 2, 2)
    # pairwise tree keeps it fast and stable
    while mats.shape[0] > 1:
        n = mats.shape[0]
        even = mats[0:n - 1:2]
        odd = mats[1:n:2]
        comb = _lse_combine(even, odd)
        if n % 2 == 1:
            comb = np.concatenate([comb, mats[n - 1:n]], axis=0)
        mats = comb
    total = np.logaddexp.reduce(mats.reshape(-1))
    # the single t=0 pad step (p=19) carries the -BSH shift: add it back
    gold += BSH
    total += BSH
    return np.stack([gold, total]).astype(np.float32)


if __name__ == "__main__":
    rng = np.random.default_rng(0)
    demo = dict(
        emission_scores=rng.standard_normal((T, 2)).astype(np.float32),
        label=rng.integers(0, 2, T),
        who2who_state=np.concatenate([[2], rng.integers(0, 2, T - 1)]),
        position_state=np.concatenate([[19], rng.integers(0, 19, T - 1)]),
        who2who_params=rng.standard_normal((2, 2, 2)).astype(np.float32),
        position_params=rng.standard_normal((19, 2, 2)).astype(np.float32),
    )
    print(kernel(**demo))


# revision 43
# speedup vs baseline: 1.0585x; 1.0409x over previous
"""Trainium2 Bass kernel for a 2-state linear-chain CRF loss (BiLSTM-CRF loss_fn).

Computes, for a single conversation of length T = 2,097,152:
  gold_score  = sum_t em[t, lab[t]] + sum_{t>0} trans[t][lab[t-1], lab[t]]
  total_score = logsumexp of the CRF forward recursion
where trans[t] = who2who_sub[w[t]] + position_sub[p[t]] (60 possible 2x2
matrices; indices 2/19 select an all-zero padding matrix).

Design (one NeuronCore per contiguous chunk of 262,144 steps, 8 cores):

* Per-step matrices: trans+em is built as 4 fp16 streams by per-class masked
  accumulation (19 position classes + 2 who2who classes + emission fold).
  Class supports are disjoint, so sums of masked values are exact in fp16;
  the masked values are combined PAIRWISE (a small in-group tree) so the
  per-stream dependency depth is ~8 instead of 21 serial adds.  Work is
  split three ways: DVE runs fused (idx==c)*V tensor_scalars (4x fp16 mode)
  plus most adds; the ACT engine produces masked values for a suffix of
  position classes as Relu((V+B) - (V+B)*(p-c)^2) with B=4 making the peak
  positive (the spurious +B*[p>=a] is removed by one (p>a-.5)*B mask and 4
  subtracts; the t=0 pad step ends shifted by exactly -B, corrected on the
  host); GPSIMD takes a striped share of the adds.

* Gold score: the label-pair stream msel = 2*lab[t-1]+lab[t] selects one of
  the 4 finished streams per step; gold = sum_t acc[msel_t][t] via 4
  is_equal masks + multiply + accum_out per sub-chunk.  Exactness: stream
  values are single-fp16-rounded table values (+ exact-in-fp16 shifts), and
  the host chooses each table entry's fp16 rounding DIRECTION (greedy sign
  optimization over the 19x2x4 reachable cells) so the systematic selection
  bias cancels to ~1e-5 relative.

* Forward pass: the recursion is a product of 2x2 matrices in the (log, +)
  semiring; each core tree-reduces with LSE(a,b) = a + ln(1+exp(b-a)) on
  ACT.  The chunk is split into 3 sub-chunks of 1024/512/512 steps per
  partition so each sub-chunk's tree overlaps the next one's stream build
  and only the last (small) tree is exposed at the end.  Each core ships
  its 3*128 sub-chunk matrices + per-partition gold; the host does the
  O(cores*P) ordered log-semiring combine (vectorized numpy).

* All inputs ship as one fp16 blob [par | p | w | msel | em0 | em1] in 3
  DMAs so the class masks start immediately.
"""

from contextlib import ExitStack

import numpy as np

import concourse.bass as bass
import concourse.bacc as bacc
import concourse.mybir as mybir
import concourse.tile as tile
from concourse import bass_utils

dt = mybir.dt
ALU = mybir.AluOpType
AF = mybir.ActivationFunctionType
AX = mybir.AxisListType

T = 2097152
NCORES = 8
P = 128                  # SBUF partitions
L = T // NCORES          # steps per core = 262144
F = L // P               # steps per partition = 2048
SC_SIZES = (1024, 512, 512)
SC = len(SC_SIZES)
NPOS = 19                # position classes with nonzero matrices
BSH = 8.0                # ACT positivity shift
ACT_LO = 9               # position classes >= this use ACT-produced mv
EW = 4 * NT + 1          # out row: NT matrices (4 entries each) + gold

# param row layout (f32 words): [pos' 19*4 | D 4 | VB 19*4 | negVB 19*4]
# pos' = pos + w2w[1] (folded);  D = w2w[0] - w2w[1]
NPAR = 19 * 4 + 4 + 19 * 4 + 19 * 4
COL_POS = 0
COL_W = 76
COL_VB = 80
COL_NVB = 156

W0 = 2 * NPAR + 5 * F    # fp16 blob columns


_NC_CACHE = None
LAST_RESULTS = None  # BassKernelResults of the most recent kernel() call


def _comp(i, j):
    return i * 2 + j


def _build_nc():
    nc = bacc.Bacc()

    b0_d = nc.dram_tensor("blob0", [P, W0], dt.float16, kind="ExternalInput")
    out_d = nc.dram_tensor("out", [P, EW], dt.float32, kind="ExternalOutput")

    with ExitStack() as ctx:
        tc = ctx.enter_context(tile.TileContext(nc))
        pool = ctx.enter_context(tc.tile_pool(name="main", bufs=1))

        # ---- loads: [par | p | w | msel | em0 | em1] in 3 DMAs ----
        b0 = pool.tile([P, W0], dt.float16, tag="b0", name="b0")
        hq = 2 * NPAR + 1024       # par + p columns for sub-chunk 0
        h0 = 2 * NPAR + F          # par + p
        h1 = h0 + 2 * F            # + w + msel
        nc.sync.dma_start(b0[:, 0:hq], b0_d[:, 0:hq])
        nc.sync.dma_start(b0[:, hq:h0], b0_d[:, hq:h0])
        nc.sync.dma_start(b0[:, h0:h1], b0_d[:, h0:h1])
        nc.sync.dma_start(b0[:, h1:W0], b0_d[:, h1:W0])

        par32 = b0[:, 0:2 * NPAR].bitcast(dt.float32)
        p_t = b0[:, 2 * NPAR:h0]
        w_t = b0[:, h0:h0 + F]
        msel_t = b0[:, h0 + F:h1]
        em0_t = b0[:, h1:h1 + F]
        em1_t = b0[:, h1 + F:W0]

        def V(col):
            return par32[:, col:col + 1]

        bias_c = {}
        for c in range(ACT_LO, NPOS):
            t_ = pool.tile([P, 1], dt.float32, tag=f"bc{c}", name=f"bc{c}")
            nc.vector.memset(t_[:], -float(c))
            bias_c[c] = t_
        bconst = pool.tile([P, 1], dt.float32, tag="bconst", name="bconst")
        nc.vector.memset(bconst[:], BSH)

        FSMAX = max(SC_SIZES)
        HSMAX = FSMAX // 2
        SC_OFF = [sum(SC_SIZES[:i]) for i in range(SC)]
        acc = [
            pool.tile([P, F], dt.float16, tag=f"acc{c}", name=f"acc{c}")
            for c in range(4)
        ]
        # mv work tiles: 2 alternating sets of 3 members per comp, sized to
        # one sub-chunk; sets alternate per group so production of group g+1
        # never write-after-read blocks on group g's consumption
        mv = [
            pool.tile([P, FSMAX], dt.float16, tag=f"mv{i}", name=f"mv{i}")
            for i in range(2 * 3 * 4)
        ]

        def mv_t(gi, ki, c, s):
            t = mv[((gi % 2) * 3 + ki) * 4 + c]
            return t[:, 0:SC_SIZES[s]]

        amv = [
            pool.tile([P, FSMAX], dt.float16, tag=f"amv{i}", name=f"amv{i}")
            for i in range(2 * 3 * 4)
        ]

        def amv_t(gi, ki, c, s):
            t = amv[((gi % 2) * 3 + ki) * 4 + c]
            return t[:, 0:SC_SIZES[s]]

        sqt = [
            pool.tile([P, F], dt.float16, tag=f"sq{i}", name=f"sq{i}")
            for i in range(2)
        ]
        mvb = pool.tile([P, FSMAX], dt.float16, tag="mvb", name="mvb")
        qm = [
            pool.tile([P, FSMAX], dt.float16, tag=f"qm{i}", name=f"qm{i}")
            for i in range(2)
        ]
        gprod = [
            pool.tile([P, FSMAX], dt.float16, tag=f"gp{i}", name=f"gp{i}")
            for i in range(2)
        ]
        gcell = pool.tile([P, 4 * NT], dt.float32, tag="gcell", name="gcell")
        res = pool.tile([P, EW], dt.float32, tag="res", name="res")

        # striped DVE/Pool assignment for accumulate adds
        POOL_NUM, POOL_DEN = 1, 3
        add_ctr = [0] * 4

        def add_eng(comp):
            add_ctr[comp] += 1
            k = (add_ctr[comp] * 3 + comp) % POOL_DEN
            return nc.gpsimd if k < POOL_NUM else nc.vector

        def tadd(comp, out, a, b):
            add_eng(comp).tensor_add(out, a, b)

        def sc_views(s, h=0, nh=1):
            sz = SC_SIZES[s] // nh
            lo = SC_OFF[s] + h * sz
            sl = slice(lo, lo + sz)
            return (p_t[:, sl], w_t[:, sl], msel_t[:, sl],
                    [a[:, sl] for a in acc], sl)

        dve_classes = list(range(0, ACT_LO))
        dgroups = [dve_classes[i:i + 3]
                   for i in range(0, len(dve_classes), 3)]
        act_classes = list(range(ACT_LO, NPOS))
        agroups = [act_classes[i:i + 3]
                   for i in range(0, len(act_classes), 3)]

        def emit_dve_group(s, gi):
            grp = dgroups[gi]
            p_s, w_s, m_s, acc_s, sl = sc_views(s)
            FS = SC_SIZES[s]
            for c in range(4):
                tiles = []
                for ki, k in enumerate(grp):
                    m = mv_t(gi, ki, c, s)
                    nc.vector.tensor_scalar(
                        m, p_s, float(k), V(COL_POS + 4 * k + c),
                        ALU.is_equal, ALU.mult,
                    )
                    tiles.append(m)
                if len(tiles) >= 3:
                    tadd(c, tiles[0], tiles[0], tiles[1])
                    rest = tiles[2]
                    if gi == 0:
                        tadd(c, acc_s[c], tiles[0], rest)
                    else:
                        tadd(c, tiles[0], tiles[0], rest)
                        tadd(c, acc_s[c], acc_s[c], tiles[0])
                else:
                    while len(tiles) > 1:
                        tadd(c, tiles[0], tiles[0], tiles[1])
                        tiles = [tiles[0]] + tiles[2:]
                    if gi == 0:
                        nc.vector.tensor_copy(acc_s[c], tiles[0])
                    else:
                        tadd(c, acc_s[c], acc_s[c], tiles[0])

        def emit_act_group(s, gi):
            grp = agroups[gi]
            p_s, w_s, m_s, acc_s, sl = sc_views(s)
            FS = SC_SIZES[s]
            for ki, k in enumerate(grp):
                sq = sqt[ki % 2][:, SC_OFF[s]:SC_OFF[s] + FS]
                nc.scalar.activation(sq, p_s, AF.Square, bias=bias_c[k][:])
                for c in range(4):
                    nc.scalar.activation(
                        amv_t(gi, ki, c, s), sq, AF.Relu,
                        bias=V(COL_VB + 4 * k + c),
                        scale=V(COL_NVB + 4 * k + c),
                    )
            for c in range(4):
                tiles = [amv_t(gi, ki, c, s) for ki in range(len(grp))]
                while len(tiles) > 1:
                    tadd(c, tiles[0], tiles[0], tiles[1])
                    tiles = [tiles[0]] + tiles[2:]
                tadd(c, acc_s[c], acc_s[c], tiles[0])

        def emit_sub_w_em(s, h=0, nh=1):
            p_s, w_s, m_s, acc_s, sl = sc_views(s, h, nh)
            FS = SC_SIZES[s] // nh
            lo = h * FS
            # remove the spurious +B over [p >= ACT_LO] (includes the t=0
            # pad step p=19; host adds B back to both outputs)
            nc.vector.tensor_scalar(
                mvb[:, lo:lo + FS], p_s, ACT_LO - 0.5, bconst[:], ALU.is_gt,
                ALU.mult,
            )
            for c in range(4):
                add_eng(c).tensor_sub(acc_s[c], acc_s[c], mvb[:, lo:lo + FS])
            # who2who: w2w[1] is folded into the position table, so only
            # (w==0)*(w2w[0]-w2w[1]) remains (w==2 occurs only at t=0)
            for c in range(4):
                m0 = mv[0 * 4 + c][:, lo:lo + FS]
                nc.vector.tensor_scalar(
                    m0, w_s, 0.0, V(COL_W + c), ALU.is_equal, ALU.mult,
                )
                tadd(c, acc_s[c], acc_s[c], m0)
            # emission fold: acc[i,j] += em_j
            for c in range(4):
                em_s = (em0_t if c % 2 == 0 else em1_t)[:, sl]
                tadd(c, acc_s[c], acc_s[c], em_s)

        def emit_gold_pair(s, tidx, h, nh, pair):
            p_s, w_s, m_s, acc_s, sl = sc_views(s, h, nh)
            FS = SC_SIZES[s] // nh
            lo = h * FS
            q = qm[pair % 2][:, lo:lo + FS]
            nc.vector.tensor_scalar(q, m_s, float(pair), None, ALU.is_equal)
            pr = gprod[pair % 2][:, lo:lo + FS]
            nc.vector.tensor_mul(pr, q, acc_s[pair])
            nc.vector.tensor_scalar(
                pr, pr, 1.0, None, ALU.mult, ALU.add,
                accum_out=gcell[:, tidx * 4 + pair:tidx * 4 + pair + 1],
            )

        def emit_gold(s, tidx, h=0, nh=1):
            for pair in range(4):
                emit_gold_pair(s, tidx, h, nh, pair)

        def u2(ap):
            return ap.unsqueeze(2).unsqueeze(3)

        HSMAX = max(SC_SIZES) // 2
        FP16_LEVELS = 4

        def tree_scratch(setid, cap):
            def t(tag, shape, dtp):
                return pool.tile(shape, dtp, tag=f"{tag}_{setid}",
                                 name=f"{tag}_{setid}")
            return {
                "cap": cap,
                "sid": setid,
                "X16": t("X16", [P, cap, 2, 2], dt.float16),
                "Y16a": t("Y16a", [P, cap, 2, 2], dt.float16),
                "Y16b": t("Y16b", [P, cap // 2, 2, 2], dt.float16),
                "X32": t("X32", [P, max(cap // 16, 1), 2, 2], dt.float32),
                "Y0": t("Y0", [P, cap, 2, 2], dt.float32),
                "Y1": t("Y1", [P, cap // 2, 2, 2], dt.float32),
            }

        def emit_trees(specs, fill_ops=()):
            """Emit one or more same-size trees with level-interleaved
            instruction order so concurrent trees fill each other's
            dependency gaps.  fill_ops are independent thunks emitted one
            per level round to plug in-order engine stalls."""
            fill_ops = list(fill_ops)
            sts = []
            for (s, h, nh, tidx, scr) in specs:
                _, _, _, acc_s, sl = sc_views(s, h, nh)
                HS = (SC_SIZES[s] // nh) // 2
                sts.append(dict(acc=acc_s, HS=HS, tidx=tidx, scr=scr))
            # level 1 from the acc streams
            for st in sts:
                scr, HS, acc_s = st["scr"], st["HS"], st["acc"]
                X16 = scr["X16"][:, 0:HS]
                Y16a = scr["Y16a"][:, 0:HS]
                for i in range(2):
                    for j in range(2):
                        add_eng(_comp(i, j)).tensor_add(
                            X16[:, :, i:i + 1, j:j + 1],
                            u2(acc_s[_comp(i, 0)][:, 0::2]),
                            u2(acc_s[_comp(0, j)][:, 1::2]),
                        )
                        add_eng(_comp(i, j)).tensor_add(
                            Y16a[:, :, i:i + 1, j:j + 1],
                            u2(acc_s[_comp(i, 1)][:, 0::2]),
                            u2(acc_s[_comp(1, j)][:, 1::2]),
                        )
            for st in sts:
                scr, HS = st["scr"], st["HS"]
                X16, Y16a, Y0 = (scr["X16"][:, 0:HS], scr["Y16a"][:, 0:HS],
                                 scr["Y0"][:, 0:HS])
                nc.vector.tensor_sub(Y16a[:], Y16a[:], X16[:])
                nc.scalar.activation(Y0[:], Y16a[:], AF.Exp)
                nc.scalar.activation(Y0[:], Y0[:], AF.Ln, bias=1.0)
                mlev = pool.tile([P, scr["cap"], 2, 2], dt.float16,
                                 tag=f"m1_s{scr['sid']}",
                                 name=f"m1_s{scr['sid']}")[:, 0:HS]
                nc.vector.tensor_add(mlev[:], X16[:], Y0[:])
                st["mlev"] = mlev
                st["w"] = HS
                st["lev"] = 1
            while any(st["w"] > 1 for st in sts):
                if fill_ops:
                    fill_ops.pop(0)()
                for st in sts:
                    if st["w"] <= 1:
                        continue
                    scr = st["scr"]
                    mlev, w_cur = st["mlev"], st["w"]
                    w2 = w_cur // 2
                    lev = st["lev"] + 1
                    sh = [P, w2, 2, 2]
                    a_i0 = mlev[:, 0:w_cur:2, :, 0:1].broadcast_to(sh)
                    a_i1 = mlev[:, 0:w_cur:2, :, 1:2].broadcast_to(sh)
                    b_0j = mlev[:, 1:w_cur:2, 0:1, :].broadcast_to(sh)
                    b_1j = mlev[:, 1:w_cur:2, 1:2, :].broadcast_to(sh)
                    sp = (scr["Y0"] if lev % 2 == 1 else scr["Y1"])[:, 0:w2]
                    if lev <= FP16_LEVELS:
                        xv = scr["X16"][:, 0:w2]
                        yv = (scr["Y16a"] if lev % 2 == 1
                              else scr["Y16b"])[:, 0:w2]
                        (add_eng(lev % 4) if w2 >= 64 else nc.vector
                         ).tensor_add(xv, a_i0, b_0j)
                        (add_eng((lev + 2) % 4) if w2 >= 64 else nc.vector
                         ).tensor_add(yv, a_i1, b_1j)
                        nc.vector.tensor_sub(yv, yv, xv)
                        nc.scalar.activation(sp, yv, AF.Exp)
                    else:
                        xv = scr["X32"][:, 0:w2]
                        yv = sp
                        nc.vector.tensor_add(xv, a_i0, b_0j)
                        nc.vector.tensor_add(yv, a_i1, b_1j)
                        nc.vector.tensor_sub(yv, yv, xv)
                        nc.scalar.activation(sp, sp, AF.Exp)
                    nc.scalar.activation(sp, sp, AF.Ln, bias=1.0)
                    mdt = dt.float16 if lev <= FP16_LEVELS else dt.float32
                    mwidth = max(scr["cap"] // (2 ** (lev - 1)), 1)
                    mnext = pool.tile(
                        [P, mwidth, 2, 2], mdt,
                        tag=f"m{lev}_s{scr['sid']}",
                        name=f"m{lev}_s{scr['sid']}")[:, 0:w2]
                    nc.vector.tensor_add(mnext[:], xv, sp)
                    st["mlev"] = mnext
                    st["w"] = w2
                    st["lev"] = lev
            for fn in fill_ops:
                fn()
            for st in sts:
                nc.vector.tensor_copy(
                    res[:, 4 * st["tidx"]:4 * st["tidx"] + 4].rearrange(
                        "p (a b c) -> p a b c", a=1, b=2),
                    st["mlev"][:],
                )

        scr0 = tree_scratch(0, HSMAX)
        scr1 = tree_scratch(1, HSMAX // 2)

        # ---- skewed block emission: sub-chunk s trails s-1 by SKEW blocks
        # so completions stagger; the LAST sub-chunk's tail (links, gold,
        # tree) is split into two concurrent halves to break the final
        # serial LSE chain ----
        SKEW = 6
        blocks = []
        for s in range(SC):
            seq = []
            seq.append(lambda s=s: emit_dve_group(s, 0))
            for gi in range(len(agroups)):
                seq.append(lambda s=s, gi=gi: emit_act_group(s, gi))
                if gi + 1 < len(dgroups):
                    seq.append(lambda s=s, gi=gi: emit_dve_group(s, gi + 1))
            if s < SC - 1:
                seq.append(lambda s=s: emit_sub_w_em(s))
                seq.append(lambda s=s: emit_gold(s, s))
                seq.append(lambda s=s: emit_trees([(s, 0, 1, s, scr0)]))
            else:
                seq.append(lambda s=s: emit_sub_w_em(s, 0, 2))
                seq.append(lambda s=s: emit_sub_w_em(s, 1, 2))

                def gold_fills(s=s):
                    ops = []
                    for h in range(2):
                        for pair in range(4):
                            ops.append(lambda s=s, h=h, pair=pair:
                                       emit_gold_pair(s, s + h, h, 2, pair))
                    return ops

                seq.append(lambda s=s: emit_trees(
                    [(s, 0, 2, s, scr0), (s, 1, 2, s + 1, scr1)],
                    fill_ops=gold_fills(s)))
            for bi, fn in enumerate(seq):
                blocks.append((bi + SKEW * s, s, fn))
        blocks.sort(key=lambda kv: (kv[0], kv[1]))
        for _, _, fn in blocks:
            fn()

    nc.compile()

    # Exp/Ln/Square/Relu all live in 'natural_log_exp_and_others', but
    # insert_act_table_loads picks the first set containing each function,
    # emitting alternating table reloads (1.3 us each).  Retarget every load
    # to the combined set and drop the now-redundant ones.
    from concourse.hw_specs import get_activation_tables

    tables = list(get_activation_tables(nc.m.arch).keys())
    combined = tables.index("natural_log_exp_and_others")
    for b in nc.bb_map.values():
        insts = b.bb.instructions
        kept = []
        seen_load = False
        for ins in insts:
            if ins.opcode == "LoadActFuncSet":
                si = ins.sync_info
                assert not (si and (si.on_wait or si.on_update)), ins.name
                if seen_load:
                    continue
                ins.act_func_set_id = combined
                seen_load = True
            kept.append(ins)
        if len(kept) != len(insts):
            b.bb.instructions = kept
    return nc


def _get_nc():
    global _NC_CACHE
    if _NC_CACHE is None:
        _NC_CACHE = _build_nc()
    return _NC_CACHE


def _f16_candidates(x, grid_pow=None):
    """Nearest fp16 (or 2^grid_pow-grid) value and its other-side neighbor."""
    if grid_pow is None:
        lo = np.float16(x)
        res = float(x) - float(lo)
        if res == 0.0:
            return np.float32(lo), np.float32(lo)
        hi = np.nextafter(lo, np.float16(np.inf if res > 0 else -np.inf),
                          dtype=np.float16)
        return np.float32(lo), np.float32(hi)
    g = 2.0 ** grid_pow
    lo = np.floor(float(x) / g) * g
    hi = lo + g
    if abs(float(x) - lo) <= abs(hi - float(x)):
        return np.float32(lo), np.float32(hi)
    return np.float32(hi), np.float32(lo)


def _optimize_tables(pos, w2w):
    """fp16 tables with per-entry rounding direction chosen so the
    systematic selected-sum bias over the reachable (p,w) cells cancels.
    w2w[1] is folded into the position table (P' = pos + w2w[1]); the
    residual class value is D = w2w[0] - w2w[1].  ACT-suffix rows sit on
    the 2^-7 grid so V+BSH stays fp16-exact."""
    posr = pos.reshape(NPOS, 4).astype(np.float64)
    wr = w2w.reshape(2, 4).astype(np.float64)
    Pp = posr + wr[1]
    Dv = wr[0] - wr[1]

    Pc = np.zeros((NPOS, 4, 2), np.float32)
    for k in range(NPOS):
        gp = -7 if k >= ACT_LO else None
        for c in range(4):
            Pc[k, c] = _f16_candidates(Pp[k, c], gp)
    Dc = np.zeros((4, 2), np.float32)
    for c in range(4):
        Dc[c] = _f16_candidates(Dv[c])

    # cell (k, w=1): value = P16[k];  cell (k, w=0): fp16(P16[k] + D16)
    e1 = posr + wr[1]
    e0 = posr + wr[0]
    d1 = Pc.astype(np.float64) - e1[:, :, None]               # [19,4,2]
    d0 = ((Pc[:, :, :, None].astype(np.float16)
           + Dc[None, :, None, :].astype(np.float16)).astype(np.float16)
          .astype(np.float64) - e0[:, :, None, None])          # [19,4,2,2]

    Ps = np.zeros((NPOS, 4), np.intp)
    Ds = np.zeros(4, np.intp)

    def total():
        s = 0.0
        for k in range(NPOS):
            for c in range(4):
                s += d1[k, c, Ps[k, c]] + d0[k, c, Ps[k, c], Ds[c]]
        return s

    best = total()
    for _ in range(4):
        improved = False
        for k in range(NPOS):
            for c in range(4):
                Ps[k, c] ^= 1
                t2 = total()
                if abs(t2) < abs(best):
                    best = t2
                    improved = True
                else:
                    Ps[k, c] ^= 1
        for c in range(4):
            Ds[c] ^= 1
            t2 = total()
            if abs(t2) < abs(best):
                best = t2
                improved = True
            else:
                Ds[c] ^= 1
        if not improved:
            break

    P16 = np.take_along_axis(Pc, Ps[:, :, None], axis=2)[:, :, 0]
    D16 = Dc[np.arange(4), Ds]
    return P16.astype(np.float32), D16.astype(np.float32)


def _lse_combine(A, B):
    """ordered log-semiring 2x2 product, vectorized over leading dims"""
    return np.logaddexp(A[..., :, 0:1] + B[..., 0:1, :],
                        A[..., :, 1:2] + B[..., 1:2, :])


def kernel(**inputs):
    em = np.asarray(inputs["emission_scores"], dtype=np.float32)
    lab = np.asarray(inputs["label"]).astype(np.float32)
    w = np.asarray(inputs["who2who_state"]).astype(np.float32)
    p = np.asarray(inputs["position_state"]).astype(np.float32)
    w2w = np.asarray(inputs["who2who_params"], dtype=np.float32)
    pos = np.asarray(inputs["position_params"], dtype=np.float32)
    assert em.shape == (T, 2), em.shape

    labp = np.empty_like(lab)
    labp[0] = 0.0
    labp[1:] = lab[:-1]
    msel = (2.0 * labp + lab).astype(np.float16)

    P16, D16 = _optimize_tables(pos, w2w)
    vb = P16 + np.float32(BSH)   # fp16-exact for the ACT rows (2^-7 grid)
    par_row = np.concatenate([
        P16.reshape(-1), D16.reshape(-1), vb.reshape(-1), (-vb).reshape(-1)
    ]).astype(np.float32)
    assert par_row.shape[0] == NPAR
    par16 = np.broadcast_to(par_row.view(np.float16), (P, 2 * NPAR))

    p16 = p.astype(np.float16)
    w16 = w.astype(np.float16)
    em16 = em.astype(np.float16)

    in_maps = []
    for k in range(NCORES):
        sl = slice(k * L, (k + 1) * L)
        blob0 = np.concatenate(
            [
                par16,
                p16[sl].reshape(P, F),
                w16[sl].reshape(P, F),
                msel[sl].reshape(P, F),
                np.ascontiguousarray(em16[sl, 0].reshape(P, F)),
                np.ascontiguousarray(em16[sl, 1].reshape(P, F)),
            ],
            axis=1,
        )
        in_maps.append({"blob0": np.ascontiguousarray(blob0)})

    nc = _get_nc()
    kr = bass_utils.run_bass_kernel_spmd(nc, in_maps, core_ids=list(range(NCORES)))
    global LAST_RESULTS
    LAST_RESULTS = kr
    results = kr.results

    # host combine: ordered product of NCORES*P*SC 2x2 matrices + gold sum
    rows = np.stack([np.asarray(r["out"], dtype=np.float64) for r in results])
    gold = rows[:, :, 4 * NT].sum()
    mats = rows[:, :, 0:4 * NT].reshape(NCORES * P * NT, 2, 2)
    # pairwise tree keeps it fast and stable
    while mats.shape[0] > 1:
        n = mats.shape[0]
        even = mats[0:n - 1:2]
        odd = mats[1:n:2]
        comb = _lse_combine(even, odd)
        if n % 2 == 1:
            comb = np.concatenate([comb, mats[n - 1:n]], axis=0)
        mats = comb
    total = np.logaddexp.reduce(mats.reshape(-1))
    # the single t=0 pad step (p=19) carries the -BSH shift: add it back
    gold += BSH
    total += BSH
    return np.stack([gold, total]).astype(np.float32)


if __name__ == "__main__":
    rng = np.random.default_rng(0)
    demo = dict(
        emission_scores=rng.standard_normal((T, 2)).astype(np.float32),
        label=rng.integers(0, 2, T),
        who2who_state=np.concatenate([[2], rng.integers(0, 2, T - 1)]),
        position_state=np.concatenate([[19], rng.integers(0, 19, T - 1)]),
        who2who_params=rng.standard_normal((2, 2, 2)).astype(np.float32),
        position_params=rng.standard_normal((19, 2, 2)).astype(np.float32),
    )
    print(kernel(**demo))


# revision 44
# speedup vs baseline: 1.0653x; 1.0064x over previous
"""Trainium2 Bass kernel for a 2-state linear-chain CRF loss (BiLSTM-CRF loss_fn).

Computes, for a single conversation of length T = 2,097,152:
  gold_score  = sum_t em[t, lab[t]] + sum_{t>0} trans[t][lab[t-1], lab[t]]
  total_score = logsumexp of the CRF forward recursion
where trans[t] = who2who_sub[w[t]] + position_sub[p[t]] (60 possible 2x2
matrices; indices 2/19 select an all-zero padding matrix).

Design (one NeuronCore per contiguous chunk of 262,144 steps, 8 cores):

* Per-step matrices: trans+em is built as 4 fp16 streams by per-class masked
  accumulation (19 position classes + 2 who2who classes + emission fold).
  Class supports are disjoint, so sums of masked values are exact in fp16;
  the masked values are combined PAIRWISE (a small in-group tree) so the
  per-stream dependency depth is ~8 instead of 21 serial adds.  Work is
  split three ways: DVE runs fused (idx==c)*V tensor_scalars (4x fp16 mode)
  plus most adds; the ACT engine produces masked values for a suffix of
  position classes as Relu((V+B) - (V+B)*(p-c)^2) with B=4 making the peak
  positive (the spurious +B*[p>=a] is removed by one (p>a-.5)*B mask and 4
  subtracts; the t=0 pad step ends shifted by exactly -B, corrected on the
  host); GPSIMD takes a striped share of the adds.

* Gold score: the label-pair stream msel = 2*lab[t-1]+lab[t] selects one of
  the 4 finished streams per step; gold = sum_t acc[msel_t][t] via 4
  is_equal masks + multiply + accum_out per sub-chunk.  Exactness: stream
  values are single-fp16-rounded table values (+ exact-in-fp16 shifts), and
  the host chooses each table entry's fp16 rounding DIRECTION (greedy sign
  optimization over the 19x2x4 reachable cells) so the systematic selection
  bias cancels to ~1e-5 relative.

* Forward pass: the recursion is a product of 2x2 matrices in the (log, +)
  semiring; each core tree-reduces with LSE(a,b) = a + ln(1+exp(b-a)) on
  ACT.  The chunk is split into 3 sub-chunks of 1024/512/512 steps per
  partition so each sub-chunk's tree overlaps the next one's stream build
  and only the last (small) tree is exposed at the end.  Each core ships
  its 3*128 sub-chunk matrices + per-partition gold; the host does the
  O(cores*P) ordered log-semiring combine (vectorized numpy).

* All inputs ship as one fp16 blob [par | p | w | msel | em0 | em1] in 3
  DMAs so the class masks start immediately.
"""

from contextlib import ExitStack

import numpy as np

import concourse.bass as bass
import concourse.bacc as bacc
import concourse.mybir as mybir
import concourse.tile as tile
from concourse import bass_utils

dt = mybir.dt
ALU = mybir.AluOpType
AF = mybir.ActivationFunctionType
AX = mybir.AxisListType

T = 2097152
NCORES = 8
P = 128                  # SBUF partitions
L = T // NCORES          # steps per core = 262144
F = L // P               # steps per partition = 2048
SC_SIZES = (1024, 512, 512)
SC = len(SC_SIZES)
NPOS = 19                # position classes with nonzero matrices
BSH = 8.0                # ACT positivity shift
ACT_LO = 9               # position classes >= this use ACT-produced mv
EW = 4 * NT + 1          # out row: NT matrices (4 entries each) + gold

# param row layout (f32 words): [pos' 19*4 | D 4 | VB 19*4 | negVB 19*4]
# pos' = pos + w2w[1] (folded);  D = w2w[0] - w2w[1]
NPAR = 19 * 4 + 4 + 19 * 4 + 19 * 4
COL_POS = 0
COL_W = 76
COL_VB = 80
COL_NVB = 156

W0 = 2 * NPAR + 5 * F    # fp16 blob columns


_NC_CACHE = None
LAST_RESULTS = None  # BassKernelResults of the most recent kernel() call


def _comp(i, j):
    return i * 2 + j


def _build_nc():
    nc = bacc.Bacc()

    b0_d = nc.dram_tensor("blob0", [P, W0], dt.float16, kind="ExternalInput")
    out_d = nc.dram_tensor("out", [P, EW], dt.float32, kind="ExternalOutput")

    with ExitStack() as ctx:
        tc = ctx.enter_context(tile.TileContext(nc))
        pool = ctx.enter_context(tc.tile_pool(name="main", bufs=1))

        # ---- loads: [par | p | w | msel | em0 | em1] in 3 DMAs ----
        b0 = pool.tile([P, W0], dt.float16, tag="b0", name="b0")
        hq = 2 * NPAR + 1024       # par + p columns for sub-chunk 0
        h0 = 2 * NPAR + F          # par + p
        h1 = h0 + 2 * F            # + w + msel
        nc.sync.dma_start(b0[:, 0:hq], b0_d[:, 0:hq])
        nc.sync.dma_start(b0[:, hq:h0], b0_d[:, hq:h0])
        nc.sync.dma_start(b0[:, h0:h1], b0_d[:, h0:h1])
        nc.sync.dma_start(b0[:, h1:W0], b0_d[:, h1:W0])

        par32 = b0[:, 0:2 * NPAR].bitcast(dt.float32)
        p_t = b0[:, 2 * NPAR:h0]
        w_t = b0[:, h0:h0 + F]
        msel_t = b0[:, h0 + F:h1]
        em0_t = b0[:, h1:h1 + F]
        em1_t = b0[:, h1 + F:W0]

        def V(col):
            return par32[:, col:col + 1]

        bias_c = {}
        for c in range(ACT_LO, NPOS):
            t_ = pool.tile([P, 1], dt.float32, tag=f"bc{c}", name=f"bc{c}")
            nc.vector.memset(t_[:], -float(c))
            bias_c[c] = t_
        bconst = pool.tile([P, 1], dt.float32, tag="bconst", name="bconst")
        nc.vector.memset(bconst[:], BSH)

        FSMAX = max(SC_SIZES)
        HSMAX = FSMAX // 2
        SC_OFF = [sum(SC_SIZES[:i]) for i in range(SC)]
        acc = [
            pool.tile([P, F], dt.float16, tag=f"acc{c}", name=f"acc{c}")
            for c in range(4)
        ]
        # mv work tiles: 2 alternating sets of 3 members per comp, sized to
        # one sub-chunk; sets alternate per group so production of group g+1
        # never write-after-read blocks on group g's consumption
        mv = [
            pool.tile([P, FSMAX], dt.float16, tag=f"mv{i}", name=f"mv{i}")
            for i in range(2 * 3 * 4)
        ]

        def mv_t(gi, ki, c, s):
            t = mv[((gi % 2) * 3 + ki) * 4 + c]
            return t[:, 0:SC_SIZES[s]]

        amv = [
            pool.tile([P, FSMAX], dt.float16, tag=f"amv{i}", name=f"amv{i}")
            for i in range(2 * 3 * 4)
        ]

        def amv_t(gi, ki, c, s):
            t = amv[((gi % 2) * 3 + ki) * 4 + c]
            return t[:, 0:SC_SIZES[s]]

        sqt = [
            pool.tile([P, F], dt.float16, tag=f"sq{i}", name=f"sq{i}")
            for i in range(2)
        ]
        mvb = pool.tile([P, FSMAX], dt.float16, tag="mvb", name="mvb")
        qm = [
            pool.tile([P, FSMAX], dt.float16, tag=f"qm{i}", name=f"qm{i}")
            for i in range(2)
        ]
        gprod = [
            pool.tile([P, FSMAX], dt.float16, tag=f"gp{i}", name=f"gp{i}")
            for i in range(2)
        ]
        gcell = pool.tile([P, 4 * NT], dt.float32, tag="gcell", name="gcell")
        res = pool.tile([P, EW], dt.float32, tag="res", name="res")

        # striped DVE/Pool assignment for accumulate adds
        POOL_NUM, POOL_DEN = 1, 3
        add_ctr = [0] * 4

        def add_eng(comp):
            add_ctr[comp] += 1
            k = (add_ctr[comp] * 3 + comp) % POOL_DEN
            return nc.gpsimd if k < POOL_NUM else nc.vector

        def tadd(comp, out, a, b):
            add_eng(comp).tensor_add(out, a, b)

        def sc_views(s, h=0, nh=1):
            sz = SC_SIZES[s] // nh
            lo = SC_OFF[s] + h * sz
            sl = slice(lo, lo + sz)
            return (p_t[:, sl], w_t[:, sl], msel_t[:, sl],
                    [a[:, sl] for a in acc], sl)

        dve_classes = list(range(0, ACT_LO))
        dgroups = [dve_classes[i:i + 3]
                   for i in range(0, len(dve_classes), 3)]
        act_classes = list(range(ACT_LO, NPOS))
        agroups = [act_classes[i:i + 3]
                   for i in range(0, len(act_classes), 3)]

        def emit_dve_group(s, gi):
            grp = dgroups[gi]
            p_s, w_s, m_s, acc_s, sl = sc_views(s)
            FS = SC_SIZES[s]
            for c in range(4):
                tiles = []
                for ki, k in enumerate(grp):
                    m = mv_t(gi, ki, c, s)
                    nc.vector.tensor_scalar(
                        m, p_s, float(k), V(COL_POS + 4 * k + c),
                        ALU.is_equal, ALU.mult,
                    )
                    tiles.append(m)
                if len(tiles) >= 3:
                    tadd(c, tiles[0], tiles[0], tiles[1])
                    rest = tiles[2]
                    if gi == 0:
                        tadd(c, acc_s[c], tiles[0], rest)
                    else:
                        tadd(c, tiles[0], tiles[0], rest)
                        tadd(c, acc_s[c], acc_s[c], tiles[0])
                else:
                    while len(tiles) > 1:
                        tadd(c, tiles[0], tiles[0], tiles[1])
                        tiles = [tiles[0]] + tiles[2:]
                    if gi == 0:
                        nc.vector.tensor_copy(acc_s[c], tiles[0])
                    else:
                        tadd(c, acc_s[c], acc_s[c], tiles[0])

        def emit_act_group(s, gi):
            grp = agroups[gi]
            p_s, w_s, m_s, acc_s, sl = sc_views(s)
            FS = SC_SIZES[s]
            for ki, k in enumerate(grp):
                sq = sqt[ki % 2][:, SC_OFF[s]:SC_OFF[s] + FS]
                nc.scalar.activation(sq, p_s, AF.Square, bias=bias_c[k][:])
                for c in range(4):
                    nc.scalar.activation(
                        amv_t(gi, ki, c, s), sq, AF.Relu,
                        bias=V(COL_VB + 4 * k + c),
                        scale=V(COL_NVB + 4 * k + c),
                    )
            for c in range(4):
                tiles = [amv_t(gi, ki, c, s) for ki in range(len(grp))]
                while len(tiles) > 1:
                    tadd(c, tiles[0], tiles[0], tiles[1])
                    tiles = [tiles[0]] + tiles[2:]
                tadd(c, acc_s[c], acc_s[c], tiles[0])

        def emit_sub_w_em(s, h=0, nh=1):
            p_s, w_s, m_s, acc_s, sl = sc_views(s, h, nh)
            FS = SC_SIZES[s] // nh
            lo = h * FS
            # remove the spurious +B over [p >= ACT_LO] (includes the t=0
            # pad step p=19; host adds B back to both outputs)
            nc.vector.tensor_scalar(
                mvb[:, lo:lo + FS], p_s, ACT_LO - 0.5, bconst[:], ALU.is_gt,
                ALU.mult,
            )
            for c in range(4):
                add_eng(c).tensor_sub(acc_s[c], acc_s[c], mvb[:, lo:lo + FS])
            # who2who: w2w[1] is folded into the position table, so only
            # (w==0)*(w2w[0]-w2w[1]) remains (w==2 occurs only at t=0)
            for c in range(4):
                m0 = mv[0 * 4 + c][:, lo:lo + FS]
                nc.vector.tensor_scalar(
                    m0, w_s, 0.0, V(COL_W + c), ALU.is_equal, ALU.mult,
                )
                tadd(c, acc_s[c], acc_s[c], m0)
            # emission fold: acc[i,j] += em_j
            for c in range(4):
                em_s = (em0_t if c % 2 == 0 else em1_t)[:, sl]
                tadd(c, acc_s[c], acc_s[c], em_s)

        def emit_gold_pair(s, tidx, h, nh, pair):
            # one fused op: out = (msel==pair)*acc, accum_out = sum(out)
            p_s, w_s, m_s, acc_s, sl = sc_views(s, h, nh)
            FS = SC_SIZES[s] // nh
            lo = h * FS
            pr = gprod[pair % 2][:, lo:lo + FS]
            nc.vector.scalar_tensor_tensor(
                pr, m_s, float(pair), acc_s[pair], ALU.is_equal, ALU.mult,
                accum_out=gcell[:, tidx * 4 + pair:tidx * 4 + pair + 1],
            )

        def emit_gold(s, tidx, h=0, nh=1):
            for pair in range(4):
                emit_gold_pair(s, tidx, h, nh, pair)

        def u2(ap):
            return ap.unsqueeze(2).unsqueeze(3)

        HSMAX = max(SC_SIZES) // 2
        FP16_LEVELS = 4

        def tree_scratch(setid, cap):
            def t(tag, shape, dtp):
                return pool.tile(shape, dtp, tag=f"{tag}_{setid}",
                                 name=f"{tag}_{setid}")
            return {
                "cap": cap,
                "sid": setid,
                "X16": t("X16", [P, cap, 2, 2], dt.float16),
                "Y16a": t("Y16a", [P, cap, 2, 2], dt.float16),
                "Y16b": t("Y16b", [P, cap // 2, 2, 2], dt.float16),
                "X32": t("X32", [P, max(cap // 16, 1), 2, 2], dt.float32),
                "Y0": t("Y0", [P, cap, 2, 2], dt.float32),
                "Y1": t("Y1", [P, cap // 2, 2, 2], dt.float32),
            }

        def emit_trees(specs, fill_ops=()):
            """Emit one or more same-size trees with level-interleaved
            instruction order so concurrent trees fill each other's
            dependency gaps.  fill_ops are independent thunks emitted one
            per level round to plug in-order engine stalls."""
            fill_ops = list(fill_ops)
            sts = []
            for (s, h, nh, tidx, scr) in specs:
                _, _, _, acc_s, sl = sc_views(s, h, nh)
                HS = (SC_SIZES[s] // nh) // 2
                sts.append(dict(acc=acc_s, HS=HS, tidx=tidx, scr=scr))
            # level 1 from the acc streams
            for st in sts:
                scr, HS, acc_s = st["scr"], st["HS"], st["acc"]
                X16 = scr["X16"][:, 0:HS]
                Y16a = scr["Y16a"][:, 0:HS]
                for i in range(2):
                    for j in range(2):
                        add_eng(_comp(i, j)).tensor_add(
                            X16[:, :, i:i + 1, j:j + 1],
                            u2(acc_s[_comp(i, 0)][:, 0::2]),
                            u2(acc_s[_comp(0, j)][:, 1::2]),
                        )
                        add_eng(_comp(i, j)).tensor_add(
                            Y16a[:, :, i:i + 1, j:j + 1],
                            u2(acc_s[_comp(i, 1)][:, 0::2]),
                            u2(acc_s[_comp(1, j)][:, 1::2]),
                        )
            for st in sts:
                scr, HS = st["scr"], st["HS"]
                X16, Y16a, Y0 = (scr["X16"][:, 0:HS], scr["Y16a"][:, 0:HS],
                                 scr["Y0"][:, 0:HS])
                nc.vector.tensor_sub(Y16a[:], Y16a[:], X16[:])
                nc.scalar.activation(Y0[:], Y16a[:], AF.Exp)
                nc.scalar.activation(Y0[:], Y0[:], AF.Ln, bias=1.0)
                mlev = pool.tile([P, scr["cap"], 2, 2], dt.float16,
                                 tag=f"m1_s{scr['sid']}",
                                 name=f"m1_s{scr['sid']}")[:, 0:HS]
                nc.vector.tensor_add(mlev[:], X16[:], Y0[:])
                st["mlev"] = mlev
                st["w"] = HS
                st["lev"] = 1
            while any(st["w"] > 1 for st in sts):
                if fill_ops:
                    fill_ops.pop(0)()
                for st in sts:
                    if st["w"] <= 1:
                        continue
                    scr = st["scr"]
                    mlev, w_cur = st["mlev"], st["w"]
                    w2 = w_cur // 2
                    lev = st["lev"] + 1
                    sh = [P, w2, 2, 2]
                    a_i0 = mlev[:, 0:w_cur:2, :, 0:1].broadcast_to(sh)
                    a_i1 = mlev[:, 0:w_cur:2, :, 1:2].broadcast_to(sh)
                    b_0j = mlev[:, 1:w_cur:2, 0:1, :].broadcast_to(sh)
                    b_1j = mlev[:, 1:w_cur:2, 1:2, :].broadcast_to(sh)
                    sp = (scr["Y0"] if lev % 2 == 1 else scr["Y1"])[:, 0:w2]
                    if lev <= FP16_LEVELS:
                        xv = scr["X16"][:, 0:w2]
                        yv = (scr["Y16a"] if lev % 2 == 1
                              else scr["Y16b"])[:, 0:w2]
                        (add_eng(lev % 4) if w2 >= 64 else nc.vector
                         ).tensor_add(xv, a_i0, b_0j)
                        (add_eng((lev + 2) % 4) if w2 >= 64 else nc.vector
                         ).tensor_add(yv, a_i1, b_1j)
                        nc.vector.tensor_sub(yv, yv, xv)
                        nc.scalar.activation(sp, yv, AF.Exp)
                    else:
                        xv = scr["X32"][:, 0:w2]
                        yv = sp
                        nc.vector.tensor_add(xv, a_i0, b_0j)
                        nc.vector.tensor_add(yv, a_i1, b_1j)
                        nc.vector.tensor_sub(yv, yv, xv)
                        nc.scalar.activation(sp, sp, AF.Exp)
                    nc.scalar.activation(sp, sp, AF.Ln, bias=1.0)
                    mdt = dt.float16 if lev <= FP16_LEVELS else dt.float32
                    mwidth = max(scr["cap"] // (2 ** (lev - 1)), 1)
                    mnext = pool.tile(
                        [P, mwidth, 2, 2], mdt,
                        tag=f"m{lev}_s{scr['sid']}",
                        name=f"m{lev}_s{scr['sid']}")[:, 0:w2]
                    nc.vector.tensor_add(mnext[:], xv, sp)
                    st["mlev"] = mnext
                    st["w"] = w2
                    st["lev"] = lev
            for fn in fill_ops:
                fn()
            for st in sts:
                nc.vector.tensor_copy(
                    res[:, 4 * st["tidx"]:4 * st["tidx"] + 4].rearrange(
                        "p (a b c) -> p a b c", a=1, b=2),
                    st["mlev"][:],
                )

        scr0 = tree_scratch(0, HSMAX)
        scr1 = tree_scratch(1, HSMAX // 2)

        # ---- skewed block emission: sub-chunk s trails s-1 by SKEW blocks
        # so completions stagger; the LAST sub-chunk's tail (links, gold,
        # tree) is split into two concurrent halves to break the final
        # serial LSE chain ----
        SKEW = 6
        blocks = []
        for s in range(SC):
            seq = []
            seq.append(lambda s=s: emit_dve_group(s, 0))
            for gi in range(len(agroups)):
                seq.append(lambda s=s, gi=gi: emit_act_group(s, gi))
                if gi + 1 < len(dgroups):
                    seq.append(lambda s=s, gi=gi: emit_dve_group(s, gi + 1))
            if s < SC - 1:
                seq.append(lambda s=s: emit_sub_w_em(s))
                seq.append(lambda s=s: emit_gold(s, s))
                seq.append(lambda s=s: emit_trees([(s, 0, 1, s, scr0)]))
            else:
                seq.append(lambda s=s: emit_sub_w_em(s, 0, 2))
                seq.append(lambda s=s: emit_sub_w_em(s, 1, 2))

                def gold_fills(s=s):
                    ops = []
                    for h in range(2):
                        for pair in range(4):
                            ops.append(lambda s=s, h=h, pair=pair:
                                       emit_gold_pair(s, s + h, h, 2, pair))
                    return ops

                seq.append(lambda s=s: emit_trees(
                    [(s, 0, 2, s, scr0), (s, 1, 2, s + 1, scr1)],
                    fill_ops=gold_fills(s)))
            for bi, fn in enumerate(seq):
                blocks.append((bi + SKEW * s, s, fn))
        blocks.sort(key=lambda kv: (kv[0], kv[1]))
        for _, _, fn in blocks:
            fn()

    nc.compile()

    # Exp/Ln/Square/Relu all live in 'natural_log_exp_and_others', but
    # insert_act_table_loads picks the first set containing each function,
    # emitting alternating table reloads (1.3 us each).  Retarget every load
    # to the combined set and drop the now-redundant ones.
    from concourse.hw_specs import get_activation_tables

    tables = list(get_activation_tables(nc.m.arch).keys())
    combined = tables.index("natural_log_exp_and_others")
    for b in nc.bb_map.values():
        insts = b.bb.instructions
        kept = []
        seen_load = False
        for ins in insts:
            if ins.opcode == "LoadActFuncSet":
                si = ins.sync_info
                assert not (si and (si.on_wait or si.on_update)), ins.name
                if seen_load:
                    continue
                ins.act_func_set_id = combined
                seen_load = True
            kept.append(ins)
        if len(kept) != len(insts):
            b.bb.instructions = kept
    return nc


def _get_nc():
    global _NC_CACHE
    if _NC_CACHE is None:
        _NC_CACHE = _build_nc()
    return _NC_CACHE


def _f16_candidates(x, grid_pow=None):
    """Nearest fp16 (or 2^grid_pow-grid) value and its other-side neighbor."""
    if grid_pow is None:
        lo = np.float16(x)
        res = float(x) - float(lo)
        if res == 0.0:
            return np.float32(lo), np.float32(lo)
        hi = np.nextafter(lo, np.float16(np.inf if res > 0 else -np.inf),
                          dtype=np.float16)
        return np.float32(lo), np.float32(hi)
    g = 2.0 ** grid_pow
    lo = np.floor(float(x) / g) * g
    hi = lo + g
    if abs(float(x) - lo) <= abs(hi - float(x)):
        return np.float32(lo), np.float32(hi)
    return np.float32(hi), np.float32(lo)


def _optimize_tables(pos, w2w):
    """fp16 tables with per-entry rounding direction chosen so the
    systematic selected-sum bias over the reachable (p,w) cells cancels.
    w2w[1] is folded into the position table (P' = pos + w2w[1]); the
    residual class value is D = w2w[0] - w2w[1].  ACT-suffix rows sit on
    the 2^-7 grid so V+BSH stays fp16-exact."""
    posr = pos.reshape(NPOS, 4).astype(np.float64)
    wr = w2w.reshape(2, 4).astype(np.float64)
    Pp = posr + wr[1]
    Dv = wr[0] - wr[1]

    Pc = np.zeros((NPOS, 4, 2), np.float32)
    for k in range(NPOS):
        gp = -7 if k >= ACT_LO else None
        for c in range(4):
            Pc[k, c] = _f16_candidates(Pp[k, c], gp)
    Dc = np.zeros((4, 2), np.float32)
    for c in range(4):
        Dc[c] = _f16_candidates(Dv[c])

    # cell (k, w=1): value = P16[k];  cell (k, w=0): fp16(P16[k] + D16)
    e1 = posr + wr[1]
    e0 = posr + wr[0]
    d1 = Pc.astype(np.float64) - e1[:, :, None]               # [19,4,2]
    d0 = ((Pc[:, :, :, None].astype(np.float16)
           + Dc[None, :, None, :].astype(np.float16)).astype(np.float16)
          .astype(np.float64) - e0[:, :, None, None])          # [19,4,2,2]

    Ps = np.zeros((NPOS, 4), np.intp)
    Ds = np.zeros(4, np.intp)

    def total():
        s = 0.0
        for k in range(NPOS):
            for c in range(4):
                s += d1[k, c, Ps[k, c]] + d0[k, c, Ps[k, c], Ds[c]]
        return s

    best = total()
    for _ in range(4):
        improved = False
        for k in range(NPOS):
            for c in range(4):
                Ps[k, c] ^= 1
                t2 = total()
                if abs(t2) < abs(best):
                    best = t2
                    improved = True
                else:
                    Ps[k, c] ^= 1
        for c in range(4):
            Ds[c] ^= 1
            t2 = total()
            if abs(t2) < abs(best):
                best = t2
                improved = True
            else:
                Ds[c] ^= 1
        if not improved:
            break

    P16 = np.take_along_axis(Pc, Ps[:, :, None], axis=2)[:, :, 0]
    D16 = Dc[np.arange(4), Ds]
    return P16.astype(np.float32), D16.astype(np.float32)


def _lse_combine(A, B):
    """ordered log-semiring 2x2 product, vectorized over leading dims"""
    return np.logaddexp(A[..., :, 0:1] + B[..., 0:1, :],
                        A[..., :, 1:2] + B[..., 1:2, :])


def kernel(**inputs):
    em = np.asarray(inputs["emission_scores"], dtype=np.float32)
    lab = np.asarray(inputs["label"]).astype(np.float32)
    w = np.asarray(inputs["who2who_state"]).astype(np.float32)
    p = np.asarray(inputs["position_state"]).astype(np.float32)
    w2w = np.asarray(inputs["who2who_params"], dtype=np.float32)
    pos = np.asarray(inputs["position_params"], dtype=np.float32)
    assert em.shape == (T, 2), em.shape

    labp = np.empty_like(lab)
    labp[0] = 0.0
    labp[1:] = lab[:-1]
    msel = (2.0 * labp + lab).astype(np.float16)

    P16, D16 = _optimize_tables(pos, w2w)
    vb = P16 + np.float32(BSH)   # fp16-exact for the ACT rows (2^-7 grid)
    par_row = np.concatenate([
        P16.reshape(-1), D16.reshape(-1), vb.reshape(-1), (-vb).reshape(-1)
    ]).astype(np.float32)
    assert par_row.shape[0] == NPAR
    par16 = np.broadcast_to(par_row.view(np.float16), (P, 2 * NPAR))

    p16 = p.astype(np.float16)
    w16 = w.astype(np.float16)
    em16 = em.astype(np.float16)

    in_maps = []
    for k in range(NCORES):
        sl = slice(k * L, (k + 1) * L)
        blob0 = np.concatenate(
            [
                par16,
                p16[sl].reshape(P, F),
                w16[sl].reshape(P, F),
                msel[sl].reshape(P, F),
                np.ascontiguousarray(em16[sl, 0].reshape(P, F)),
                np.ascontiguousarray(em16[sl, 1].reshape(P, F)),
            ],
            axis=1,
        )
        in_maps.append({"blob0": np.ascontiguousarray(blob0)})

    nc = _get_nc()
    kr = bass_utils.run_bass_kernel_spmd(nc, in_maps, core_ids=list(range(NCORES)))
    global LAST_RESULTS
    LAST_RESULTS = kr
    results = kr.results

    # host combine: ordered product of NCORES*P*SC 2x2 matrices + gold sum
    rows = np.stack([np.asarray(r["out"], dtype=np.float64) for r in results])
    gold = rows[:, :, 4 * NT].sum()
    mats = rows[:, :, 0:4 * NT].reshape(NCORES * P * NT, 2, 2)
    # pairwise tree keeps it fast and stable
    while mats.shape[0] > 1:
        n = mats.shape[0]
        even = mats[0:n - 1:2]
        odd = mats[1:n:2]
        comb = _lse_combine(even, odd)
        if n % 2 == 1:
            comb = np.concatenate([comb, mats[n - 1:n]], axis=0)
        mats = comb
    total = np.logaddexp.reduce(mats.reshape(-1))
    # the single t=0 pad step (p=19) carries the -BSH shift: add it back
    gold += BSH
    total += BSH
    return np.stack([gold, total]).astype(np.float32)


if __name__ == "__main__":
    rng = np.random.default_rng(0)
    demo = dict(
        emission_scores=rng.standard_normal((T, 2)).astype(np.float32),
        label=rng.integers(0, 2, T),
        who2who_state=np.concatenate([[2], rng.integers(0, 2, T - 1)]),
        position_state=np.concatenate([[19], rng.integers(0, 19, T - 1)]),
        who2who_params=rng.standard_normal((2, 2, 2)).astype(np.float32),
        position_params=rng.standard_normal((19, 2, 2)).astype(np.float32),
    )
    print(kernel(**demo))
